# revision 1
# baseline (speedup 1.0000x reference)
"""SplineConv 2-layer GNN (nn_Net_23587960389976) on 8 trn2 NeuronCores.

Structure: 5 SPMD bass launches. All value arithmetic runs on device; the
host only shards, permutes by precomputed indices, and concatenates.

  L1: H = x_shard @ [W1_0|W1_1|root1|b1-row]  -> table1 shard (bf16) + root part
  L2: per-edge gather+basis-weight messages via one-hot matmuls (layer 1)
  L3: windowed segment-sum + mean + root + ELU + GEMM2 -> table2 shard + root2
  L4: gather+weight messages (layer 2)
  L5: segment-sum + mean + root2 + log_softmax

Per-core edge schedule is made SPMD-uniform with fixed capacities:
  gather: 5 tiles of 128 slots per 128-node src chunk (640 >= Poisson(512)+5.7s)
  segsum: 18 tiles of 128 slots per 64-dst window  (2304 >= Poisson(2048)+5.7s)
"""
import sys

sys.path.insert(0, "/opt/trn_rl_repo")

import numpy as np
import ml_dtypes

import concourse.bass as bass
import concourse.mybir as mybir

BF16 = ml_dtypes.bfloat16
F32 = np.float32

N_NODES = 50000
N_EDGES = 1600000
F_IN, F_HID, F_OUT = 1433, 16, 7
N_CORES = 8
NPC = N_NODES // N_CORES           # 6250
P = 128
N_CHUNKS = (N_NODES + P - 1) // P  # 391
NPAD = 397 * P                     # 50816 (chunk-padded)
KPAD = 1536                        # 1433+1 bias row, padded to 12*128
T0 = 5                             # gather tiles per chunk
N_G = N_CHUNKS * T0                # 1955 gather tiles
NG_PAD = ((N_G + 31) // 32) * 32   # 1984 (batch multiple)
N_CHUNKS_PAD = (NG_PAD + T0 - 1) // T0  # 397
WIN = 64
N_WIN = (NPC + WIN - 1) // WIN     # 98 windows
T1 = 18                            # segsum tiles per window
N_S = N_WIN * T1                   # 1764 segsum tiles
NT1 = 49                           # L1 node tiles (49*128 = 6272)
NPC_PAD = NT1 * P                  # 6272

# ------------------------------------------------------------------ patches
import concourse.tile as tile_mod
from concourse.tile import TileContext
from concourse.vector_clock import ScopedClock


def _patched_drain_and_barrier(self, tick_clock, wait_clock):
    nc = self.nc
    probe = nc.sync.nop(nofuse=True, hint="drain_wait_probe")
    wait_clock.add_sem_waits(probe.ins, ScopedClock({None: tick_clock.global_clock}))
    si = probe.ins.sync_info
    waits = list(si.on_wait) if si is not None else []
    if len(waits) > 1:
        probe.ins.sync_info = mybir.SyncInfo(on_update=list(si.on_update),
                                             on_wait=waits[:1])
        for w in waits[1:]:
            extra = nc.sync.nop(nofuse=True, hint="drain_wait_spill")
            extra.ins.sync_info = mybir.SyncInfo(on_update=[], on_wait=[w])
    nc.sync.drain()
    nc.all_engine_barrier()
    assert self.sems is not None
    popped = nc._tile_sem_poison_stack.pop()
    assert popped is self._sem_poison
    nc.clear_and_free_semaphores(list(self.sems.allocated().values()))
    nc.all_engine_barrier()


tile_mod.TileContext._drain_and_barrier = _patched_drain_and_barrier

_orig_lower = tile_mod.TileContext._lower_ordered_insts


def _split_multi_waits(ordered):
    for insts in ordered.values():
        out = []
        for inst in insts:
            si = getattr(inst, "sync_info", None)
            waits = list(si.on_wait) if si is not None and si.on_wait else []
            if len(waits) > 1:
                for k, w in enumerate(waits[:-1]):
                    out.append(mybir.InstNoOp(
                        name=f"{inst.name}-wsplit{k}", engine=inst.engine,
                        bass_nofuse=True,
                        sync_info=mybir.SyncInfo(on_wait=[w], on_update=[])))
                inst.sync_info = mybir.SyncInfo(on_wait=[waits[-1]],
                                                on_update=list(si.on_update))
            out.append(inst)
        insts[:] = out


def _patched_lower(self, ordered):
    _split_multi_waits(ordered)
    return _orig_lower(self, ordered)


tile_mod.TileContext._lower_ordered_insts = _patched_lower

# ------------------------------------------------------------------ launches

BATCH = 32
SBATCH = 8


def build_L1():
    nc = bass.Bass()
    xT = nc.dram_tensor("xT", [KPAD, NPC_PAD], mybir.dt.bfloat16,
                        kind="ExternalInput")
    Wc = nc.dram_tensor("Wc", [KPAD, 48], mybir.dt.bfloat16,
                        kind="ExternalInput")
    table = nc.dram_tensor("table", [NPC_PAD, 32], mybir.dt.bfloat16,
                           kind="ExternalOutput")
    root = nc.dram_tensor("root", [NPC_PAD, 16], mybir.dt.float32,
                          kind="ExternalOutput")
    with TileContext(nc) as tc:
        with tc.tile_pool(name="w", bufs=1) as wpool, \
             tc.tile_pool(name="x", bufs=4) as xpool, \
             tc.tile_pool(name="o", bufs=3) as opool, \
             tc.tile_pool(name="ps", bufs=2, space="PSUM") as pspool:
            wt = wpool.tile([P, 12, 48], mybir.dt.bfloat16)
            nc.sync.dma_start(out=wt[:], in_=Wc[:].rearrange("(a p) f -> p a f", p=P))
            for t in range(NT1):
                ps = pspool.tile([P, 48], mybir.dt.float32, tag="ps")
                xt = xpool.tile([P, 12, P], mybir.dt.bfloat16, tag="xt")
                nc.sync.dma_start(
                    out=xt[:],
                    in_=xT[:, t * P:(t + 1) * P].rearrange("(a p) n -> p a n", p=P))
                for k in range(12):
                    nc.tensor.matmul(out=ps[:], lhsT=xt[:, k, :], rhs=wt[:, k, :],
                                     start=(k == 0), stop=(k == 11))
                tb = opool.tile([P, 32], mybir.dt.bfloat16, tag="tb")
                nc.scalar.copy(out=tb[:], in_=ps[:, 0:32])
                nc.sync.dma_start(out=table[t * P:(t + 1) * P, :], in_=tb[:])
                rt = opool.tile([P, 16], mybir.dt.float32, tag="rt")
                nc.vector.tensor_copy(out=rt[:], in_=ps[:, 32:48])
                nc.sync.dma_start(out=root[t * P:(t + 1) * P, :], in_=rt[:])
    return nc


def build_gather(fdim, tab_cols, mcols):
    """L2 (fdim=16, tab_cols=32, mcols=16) / L4 (fdim=7, tab_cols=16, mcols=8)."""
    nc = bass.Bass()
    table = nc.dram_tensor("table", [NPAD, tab_cols], mybir.dt.bfloat16,
                           kind="ExternalInput")
    Et = nc.dram_tensor("Et", [P, NG_PAD * P], mybir.dt.bfloat16,
                        kind="ExternalInput")
    uin = nc.dram_tensor("u", [P, NG_PAD], mybir.dt.float32, kind="ExternalInput")
    msgs = nc.dram_tensor("msgs", [P, NG_PAD * mcols], mybir.dt.bfloat16,
                          kind="ExternalOutput")
    with TileContext(nc) as tc:
        with tc.tile_pool(name="tab", bufs=1) as tpool, \
             tc.tile_pool(name="et", bufs=3) as epool, \
             tc.tile_pool(name="u", bufs=1) as upool, \
             tc.tile_pool(name="m", bufs=3) as mpool, \
             tc.tile_pool(name="d", bufs=4) as dpool, \
             tc.tile_pool(name="ps", bufs=8, space="PSUM") as pspool:
            tab = tpool.tile([P, 397, tab_cols], mybir.dt.bfloat16)
            nc.sync.dma_start(out=tab[:],
                              in_=table[:].rearrange("(a p) f -> p a f", p=P))
            ut = upool.tile([P, NG_PAD], mybir.dt.float32)
            nc.sync.dma_start(out=ut[:], in_=uin[:])
            b0t = upool.tile([P, NG_PAD], mybir.dt.float32, tag="b0t")
            nc.vector.tensor_scalar(out=b0t[:], in0=ut[:], scalar1=-1.0,
                                    scalar2=1.0, op0=mybir.AluOpType.mult,
                                    op1=mybir.AluOpType.add)
            for t0 in range(0, NG_PAD, BATCH):
                et = epool.tile([P, BATCH, P], mybir.dt.bfloat16, tag="et")
                nc.sync.dma_start(
                    out=et[:],
                    in_=Et[:, t0 * P:(t0 + BATCH) * P].rearrange(
                        "p (a c) -> p a c", c=P))
                mt = mpool.tile([P, BATCH, mcols], mybir.dt.bfloat16, tag="mt")
                for j in range(BATCH):
                    t = t0 + j
                    ps = pspool.tile([P, 2 * fdim], mybir.dt.float32, tag="ps")
                    nc.tensor.matmul(out=ps[:], lhsT=et[:, j, :],
                                     rhs=tab[:, t // T0, 0:2 * fdim],
                                     start=True, stop=True)
                    d = dpool.tile([P, fdim], mybir.dt.float32, tag="d")
                    nc.scalar.activation(d[:], ps[:, fdim:2 * fdim],
                                         mybir.ActivationFunctionType.Copy,
                                         scale=ut[:, t:t + 1])
                    # msg = b0*g0 + u*g1
                    nc.vector.scalar_tensor_tensor(
                        out=mt[:, j, 0:fdim], in0=ps[:, 0:fdim],
                        scalar=b0t[:, t:t + 1], in1=d[:],
                        op0=mybir.AluOpType.mult, op1=mybir.AluOpType.add)
                nc.scalar.dma_start(
                    out=msgs[:, t0 * mcols:(t0 + BATCH) * mcols],
                    in_=mt[:].rearrange("p a c -> p (a c)"))
    return nc


def build_segsum(fdim, mcols, layer1):
    nc = bass.Bass()
    msgs = nc.dram_tensor("msgs", [P, N_S * mcols], mybir.dt.bfloat16,
                          kind="ExternalInput")
    cin = nc.dram_tensor("c", [P, N_S], mybir.dt.float32, kind="ExternalInput")
    iotab = nc.dram_tensor("iotab", [P, SBATCH * WIN], mybir.dt.float32,
                           kind="ExternalInput")
    invd = nc.dram_tensor("invd", [WIN, N_WIN], mybir.dt.float32,
                          kind="ExternalInput")
    root = nc.dram_tensor("root", [NPC_PAD, fdim], mybir.dt.float32,
                          kind="ExternalInput")
    if layer1:
        Wc2 = nc.dram_tensor("Wc2", [17, 21], mybir.dt.bfloat16,
                             kind="ExternalInput")
        id64 = nc.dram_tensor("id64", [WIN, WIN], mybir.dt.bfloat16,
                              kind="ExternalInput")
        table2 = nc.dram_tensor("table2", [NPC_PAD, 16], mybir.dt.bfloat16,
                                kind="ExternalOutput")
        root2 = nc.dram_tensor("root2", [NPC_PAD, 7], mybir.dt.float32,
                               kind="ExternalOutput")
    else:
        out = nc.dram_tensor("out", [NPC_PAD, 8], mybir.dt.float32,
                             kind="ExternalOutput")
    with TileContext(nc) as tc:
        with tc.tile_pool(name="m", bufs=3) as mpool, \
             tc.tile_pool(name="oh", bufs=3) as opool, \
             tc.tile_pool(name="agg", bufs=1) as apool, \
             tc.tile_pool(name="sc", bufs=1) as scpool, \
             tc.tile_pool(name="tmp", bufs=1) as tmppool, \
             tc.tile_pool(name="g2", bufs=4) as g2pool, \
             tc.tile_pool(name="ps", bufs=2, space="PSUM") as pspool, \
             tc.tile_pool(name="ps2", bufs=2, space="PSUM") as ps2pool:
            aggs = apool.tile([WIN, N_WIN, fdim], mybir.dt.float32)
            invt = scpool.tile([WIN, N_WIN], mybir.dt.float32, tag="invt")
            nc.sync.dma_start(out=invt[:], in_=invd[:])
            roott = scpool.tile([WIN, N_WIN, fdim], mybir.dt.float32, tag="roott")
            nc.sync.dma_start(
                out=roott[:],
                in_=root[0:N_WIN * WIN, :].rearrange("(a p) f -> p a f", p=WIN))
            ct = scpool.tile([P, N_S], mybir.dt.float32, tag="ct")
            nc.sync.dma_start(out=ct[:], in_=cin[:])
            iot = scpool.tile([P, SBATCH, WIN], mybir.dt.float32, tag="iot")
            nc.sync.dma_start(out=iot[:],
                              in_=iotab[:].rearrange("p (a c) -> p a c", c=WIN))
            if layer1:
                w2t = scpool.tile([17, 21], mybir.dt.bfloat16, tag="w2t")
                nc.sync.dma_start(out=w2t[:], in_=Wc2[:])
                idt = scpool.tile([WIN, WIN], mybir.dt.bfloat16, tag="idt")
                nc.sync.dma_start(out=idt[:], in_=id64[:])
            # ---- streamed segment-sum
            cur_ps = None
            for t0 in range(0, N_S, SBATCH):
                nb = min(SBATCH, N_S - t0)
                mt = mpool.tile([P, SBATCH, mcols], mybir.dt.bfloat16, tag="mt")
                nc.sync.dma_start(
                    out=mt[:, 0:nb, :],
                    in_=msgs[:, t0 * mcols:(t0 + nb) * mcols].rearrange(
                        "p (a c) -> p a c", c=mcols))
                oht = opool.tile([P, SBATCH, WIN], mybir.dt.bfloat16, tag="oht")
                nc.vector.tensor_tensor(
                    out=oht[:, 0:nb, :], in0=iot[:, 0:nb, :],
                    in1=ct[:, t0:t0 + nb].to_broadcast([P, nb, WIN]),
                    op=mybir.AluOpType.is_equal)
                for j in range(nb):
                    t = t0 + j
                    w, tw = divmod(t, T1)
                    if tw == 0:
                        cur_ps = pspool.tile([WIN, fdim], mybir.dt.float32,
                                             tag="ps")
                    nc.tensor.matmul(out=cur_ps[:], lhsT=oht[:, j, :],
                                     rhs=mt[:, j, 0:fdim],
                                     start=(tw == 0), stop=(tw == T1 - 1))
                    if tw == T1 - 1:
                        nc.scalar.copy(out=aggs[:, w, :], in_=cur_ps[:])
            # ---- mean + root
            o1 = tmppool.tile([WIN, N_WIN, fdim], mybir.dt.float32, tag="o1")
            nc.vector.tensor_tensor(
                out=o1[:], in0=aggs[:],
                in1=invt[:].to_broadcast([WIN, N_WIN, fdim]),
                op=mybir.AluOpType.mult)
            nc.vector.tensor_add(out=o1[:], in0=o1[:], in1=roott[:])
            if layer1:
                # ELU
                mneg = tmppool.tile([WIN, N_WIN, fdim], mybir.dt.float32, tag="mn")
                nc.vector.tensor_scalar(out=mneg[:], in0=o1[:], scalar1=0.0,
                                        scalar2=None, op0=mybir.AluOpType.min)
                emt = tmppool.tile([WIN, N_WIN, fdim], mybir.dt.float32, tag="em")
                nc.scalar.activation(emt[:], mneg[:],
                                     mybir.ActivationFunctionType.Exp)
                rt = tmppool.tile([WIN, N_WIN, fdim], mybir.dt.float32, tag="rt")
                nc.vector.tensor_scalar(out=rt[:], in0=o1[:], scalar1=0.0,
                                        scalar2=None, op0=mybir.AluOpType.max)
                h1 = tmppool.tile([WIN, N_WIN, fdim], mybir.dt.bfloat16, tag="h1")
                nc.vector.scalar_tensor_tensor(
                    out=h1[:], in0=emt[:], scalar=-1.0, in1=rt[:],
                    op0=mybir.AluOpType.add, op1=mybir.AluOpType.add)
                # GEMM2 per window: h2 = [h1 | 1] @ Wc2
                t2 = g2pool.tile([WIN, N_WIN, 16], mybir.dt.bfloat16, tag="t2")
                r2 = g2pool.tile([WIN, N_WIN, 7], mybir.dt.float32, tag="r2")
                nc.vector.memset(t2[:], 0.0)
                for w in range(N_WIN):
                    psT = ps2pool.tile([16, WIN], mybir.dt.bfloat16, tag="psT")
                    nc.tensor.transpose(out=psT[:], in_=h1[:, w, :],
                                        identity=idt[:])
                    h1T = g2pool.tile([17, WIN], mybir.dt.bfloat16, tag="h1T")
                    nc.vector.memset(h1T[:], 1.0)
                    nc.scalar.copy(out=h1T[0:16, :], in_=psT[:])
                    ps2 = ps2pool.tile([WIN, 21], mybir.dt.float32, tag="ps2")
                    nc.tensor.matmul(out=ps2[:], lhsT=h1T[:], rhs=w2t[:],
                                     start=True, stop=True)
                    nc.scalar.copy(out=t2[:, w, 0:14], in_=ps2[:, 0:14])
                    nc.vector.tensor_copy(out=r2[:, w, :], in_=ps2[:, 14:21])
                nc.sync.dma_start(
                    out=table2[0:N_WIN * WIN, :].rearrange("(a p) f -> p a f", p=WIN),
                    in_=t2[:])
                nc.sync.dma_start(
                    out=root2[0:N_WIN * WIN, :].rearrange("(a p) f -> p a f", p=WIN),
                    in_=r2[:])
            else:
                # log_softmax over 7 logits
                mx = tmppool.tile([WIN, N_WIN], mybir.dt.float32, tag="mx")
                nc.vector.tensor_reduce(out=mx[:], in_=o1[:],
                                        axis=mybir.AxisListType.X,
                                        op=mybir.AluOpType.max)
                z = tmppool.tile([WIN, N_WIN, fdim], mybir.dt.float32, tag="z")
                nc.vector.tensor_sub(out=z[:], in0=o1[:],
                                     in1=mx[:].to_broadcast([WIN, N_WIN, fdim]))
                ez = tmppool.tile([WIN, N_WIN, fdim], mybir.dt.float32, tag="ez")
                nc.scalar.activation(ez[:], z[:],
                                     mybir.ActivationFunctionType.Exp)
                se = tmppool.tile([WIN, N_WIN], mybir.dt.float32, tag="se")
                nc.vector.tensor_reduce(out=se[:], in_=ez[:],
                                        axis=mybir.AxisListType.X,
                                        op=mybir.AluOpType.add)
                ls = tmppool.tile([WIN, N_WIN], mybir.dt.float32, tag="ls")
                nc.scalar.activation(ls[:], se[:],
                                     mybir.ActivationFunctionType.Ln)
                ot = tmppool.tile([WIN, N_WIN, 8], mybir.dt.float32, tag="ot")
                nc.vector.memset(ot[:], 0.0)
                nc.vector.tensor_sub(out=ot[:, :, 0:7], in0=z[:],
                                     in1=ls[:].to_broadcast([WIN, N_WIN, fdim]))
                nc.sync.dma_start(
                    out=out[0:N_WIN * WIN, :].rearrange("(a p) f -> p a f", p=WIN),
                    in_=ot[:])
    return nc


# ------------------------------------------------------------------ host prep


def _rank_within_group(group_sorted):
    """group_sorted: nondecreasing group ids; returns rank of each element
    within its group."""
    n = group_sorted.shape[0]
    if n == 0:
        return np.zeros(0, dtype=np.int64)
    first = np.searchsorted(group_sorted, group_sorted, side="left")
    return np.arange(n, dtype=np.int64) - first


def plan_core(src, dst_local, u):
    E = src.shape[0]
    # gather side (src-sorted, chunked)
    og = np.argsort(src, kind="stable")
    sg = src[og]
    chunk = sg // P
    rank = _rank_within_group(chunk)
    assert rank.max(initial=0) < T0 * P, "gather chunk overflow"
    slot = chunk * (T0 * P) + rank
    slot_of_edge = np.empty(E, dtype=np.int64)
    slot_of_edge[og] = slot
    tloc = slot // P
    col = slot % P
    nloc = sg - chunk * P
    Et = np.zeros((NG_PAD, P, P), dtype=BF16)
    Et[tloc, nloc, col] = BF16(1.0)
    u_slot = np.zeros((P, NG_PAD), dtype=F32)
    u_slot[col, tloc] = u[og]
    # segsum side (dst-sorted, windowed)
    os_ = np.argsort(dst_local, kind="stable")
    ds = dst_local[os_]
    win = ds // WIN
    rank_s = _rank_within_group(win)
    assert rank_s.max(initial=0) < T1 * P, "segsum window overflow"
    pos = win * (T1 * P) + rank_s
    tloc_s = pos // P
    row = pos % P
    cvals = np.full((P, N_S), -1.0, dtype=F32)
    cvals[row, tloc_s] = (ds - win * WIN).astype(F32)
    perm = np.zeros((P, N_S), dtype=np.int64)
    perm[row, tloc_s] = slot_of_edge[os_]
    deg = np.bincount(dst_local, minlength=NPC).astype(F32)
    inv = 1.0 / np.clip(deg, 1.0, None)
    inv_pad = np.zeros(N_WIN * WIN, dtype=F32)
    inv_pad[:NPC] = inv
    inv_wl = np.ascontiguousarray(inv_pad.reshape(N_WIN, WIN).T)
    return Et, u_slot, cvals, perm, inv_wl


def _et_pmaj(Et):
    return np.ascontiguousarray(Et.transpose(1, 0, 2)).reshape(P, NG_PAD * P)


# ------------------------------------------------------------------ driver


_NC_CACHE = {}


def _get_nc(name, builder):
    if name not in _NC_CACHE:
        _NC_CACHE[name] = builder()
    return _NC_CACHE[name]


def _run(name, builder, in_maps):
    from concourse.bass_utils import run_bass_kernel_spmd
    import time
    nc = _get_nc(name, builder)
    t0 = time.time()
    res = run_bass_kernel_spmd(nc, in_maps, list(range(N_CORES)))
    _run.times[name] = time.time() - t0
    return res.results


_run.times = {}


def kernel(x, edge_attr, edge_index, W1, root1, b1, W2, root2, b2):
    x = np.asarray(x, dtype=F32)
    u = np.asarray(edge_attr, dtype=F32).reshape(-1)
    ei = np.asarray(edge_index, dtype=np.int64)
    src_all, dst_all = ei[0], ei[1]

    # --- shard edges by dst owner core
    owner = dst_all // NPC
    plans = []
    for c in range(N_CORES):
        m = owner == c
        plans.append(plan_core(src_all[m], dst_all[m] - c * NPC, u[m]))

    # --- L1: GEMM
    Wc1 = np.zeros((KPAD, 48), dtype=BF16)
    Wc1[:F_IN, 0:16] = np.asarray(W1[0], dtype=BF16)
    Wc1[:F_IN, 16:32] = np.asarray(W1[1], dtype=BF16)
    Wc1[:F_IN, 32:48] = np.asarray(root1, dtype=BF16)
    Wc1[F_IN, 32:48] = np.asarray(b1, dtype=BF16)  # bias row
    xT = np.zeros((KPAD, N_NODES), dtype=BF16)
    xT[:F_IN, :] = np.asarray(x.T, dtype=BF16)
    xT[F_IN, :] = BF16(1.0)
    in1 = []
    for c in range(N_CORES):
        sh = np.zeros((KPAD, NPC_PAD), dtype=BF16)
        sh[:, :NPC] = xT[:, c * NPC:(c + 1) * NPC]
        in1.append({"xT": sh, "Wc": Wc1})
    import os
    dbg = bool(os.environ.get("KERNEL_DEBUG"))
    r1 = _run("L1", build_L1, in1)
    if dbg:
        Wcf = np.zeros((KPAD, 48), dtype=F32); Wcf[:] = Wc1.astype(F32)
        xTf = xT.astype(F32)
        H = xTf.T @ Wcf  # [N_NODES, 48]
        t1e = H[:NPC, 0:32]
        got = r1[0]["table"][:NPC].astype(F32)
        print("L1 table relerr:", np.abs(got - t1e).max() / (np.abs(t1e).max() + 1e-9))
        re = H[:NPC, 32:48]
        print("L1 root relerr:", np.abs(r1[0]["root"][:NPC] - re).max() / (np.abs(re).max() + 1e-9))
    table1 = np.zeros((NPAD, 32), dtype=BF16)
    roots = []
    for c in range(N_CORES):
        table1[c * NPC:(c + 1) * NPC] = r1[c]["table"][:NPC]
        roots.append(np.ascontiguousarray(r1[c]["root"]))

    # --- L2: gather layer 1
    in2 = [{"table": table1, "Et": _et_pmaj(plans[c][0]),
            "u": plans[c][1]} for c in range(N_CORES)]
    r2 = _run("L2", lambda: build_gather(16, 32, 16), in2)
    if dbg:
        c = 0
        tabf = table1.astype(F32)
        m = owner == c
        s0, d0, u0 = src_all[m], dst_all[m] - c * NPC, u[m]
        Etc, usl, cv, pm, ivw = plans[c]
        got = r2[c]["msgs"].reshape(P, NG_PAD, 16).astype(F32)
        # check a few real edges
        slot_map = {}
        og = np.argsort(s0, kind="stable"); sg = s0[og]
        ch = sg // P; rk = _rank_within_group(ch); sl = ch * (T0 * P) + rk
        exp_msg = (1 - u0[og])[:, None] * tabf[sg, 0:16] + u0[og][:, None] * tabf[sg, 16:32]
        gg = got[sl % P, sl // P]
        err = np.abs(gg - exp_msg).max() / (np.abs(exp_msg).max() + 1e-9)
        print("L2 msg relerr:", err)

    # --- L3: segsum + layer-1 tail
    Wc2 = np.zeros((17, 21), dtype=BF16)
    Wc2[:16, 0:7] = np.asarray(W2[0], dtype=BF16)
    Wc2[:16, 7:14] = np.asarray(W2[1], dtype=BF16)
    Wc2[:16, 14:21] = np.asarray(root2, dtype=BF16)
    Wc2[16, 14:21] = np.asarray(b2, dtype=BF16)
    iotab = np.tile(np.arange(WIN, dtype=F32)[None, :], (P, SBATCH))
    id64 = np.eye(WIN, dtype=BF16)
    in3 = []
    for c in range(N_CORES):
        msgs = r2[c]["msgs"].reshape(P, NG_PAD, 16)
        flat = np.ascontiguousarray(msgs.transpose(1, 0, 2)).reshape(NG_PAD * P, 16)
        mp = flat[plans[c][3]]  # [P, N_S, 16]
        in3.append({"msgs": np.ascontiguousarray(mp).reshape(P, N_S * 16),
                    "c": plans[c][2], "iotab": iotab, "invd": plans[c][4],
                    "root": roots[c], "Wc2": Wc2, "id64": id64})
    r3 = _run("L3", lambda: build_segsum(16, 16, True), in3)
    table2 = np.zeros((NPAD, 16), dtype=BF16)
    roots2 = []
    for c in range(N_CORES):
        table2[c * NPC:(c + 1) * NPC] = r3[c]["table2"][:NPC]
        rr = np.zeros((NPC_PAD, 7), dtype=F32)
        rr[:] = r3[c]["root2"]
        roots2.append(rr)

    # --- L4: gather layer 2
    in4 = [{"table": table2, "Et": _et_pmaj(plans[c][0]),
            "u": plans[c][1]} for c in range(N_CORES)]
    r4 = _run("L4", lambda: build_gather(7, 16, 8), in4)

    # --- L5: segsum + final
    in5 = []
    for c in range(N_CORES):
        msgs = r4[c]["msgs"].reshape(P, NG_PAD, 8)
        flat = np.ascontiguousarray(msgs.transpose(1, 0, 2)).reshape(NG_PAD * P, 8)
        mp = flat[plans[c][3]]  # [P, N_S, 8]
        in5.append({"msgs": np.ascontiguousarray(mp).reshape(P, N_S * 8),
                    "c": plans[c][2], "iotab": iotab, "invd": plans[c][4],
                    "root": roots2[c]})
    r5 = _run("L5", lambda: build_segsum(7, 8, False), in5)

    out = np.zeros((N_NODES, F_OUT), dtype=F32)
    for c in range(N_CORES):
        out[c * NPC:(c + 1) * NPC] = r5[c]["out"][:NPC, :7]
    return out



# revision 3
# speedup vs baseline: 5.6556x; 5.6556x over previous
"""SplineConv 2-layer GNN (nn_Net_23587960389976) on 8 trn2 NeuronCores.

Structure: 5 SPMD bass launches. All value arithmetic runs on device; the
host only shards, permutes by precomputed indices, and concatenates.

  L1: H = x_shard @ [W1_0|W1_1|root1+b1row]  -> table shard (bf16) + root (f32)
  L2: per-edge basis-weighted gather via fp8 weighted-indicator matmuls:
      64-src-node chunks, lhsT column s holds (1-u_e) at row src%64 and
      u_e at row 64+src%64, rhs = [h0;h1] stacked table chunk -> msg directly.
  L3: windowed segment-sum (32-node dst windows, pure fp8 one-hot scatter
      matmuls packed 4 windows/psum partition group) + mean + root + ELU
      + GEMM2 (PE transposes + matmul, bias via K=1 ones matmul) -> table2/root2
  L4: weighted gather layer 2 (same B matrices, 7-col table)
  L5: segment-sum + mean + root2 + log_softmax

Cost-model-aware choices: matmuls are charged only out-free-size cycles, so
all gather/scatter work rides the PE; DMA is charged per-partition bytes on
the issuing engine queue, so bulk traffic is fp8 and round-robined across the
three DMA-capable queues (SP/sync, Pool/gpsimd, Act/scalar); per-instruction
vector/scalar engine overhead (~60-185ns) is amortized by batching all
DVE/Act ops over >=512-element tiles.

Per-core edge schedule is SPMD-uniform with fixed capacities:
  gather: 3 tiles of 128 slots per 64-src-chunk (384 >= max 320 on seed-0)
  segsum: 10 tiles of 128 slots per 32-dst-window (1280 >= max 1115)
"""
import sys

sys.path.insert(0, "/opt/trn_rl_repo")

import numpy as np
import ml_dtypes

import concourse.bass as bass
import concourse.mybir as mybir

BF16 = ml_dtypes.bfloat16
F8 = ml_dtypes.float8_e3m4
F32 = np.float32

N_NODES = 50000
N_EDGES = 1600000
F_IN, F_HID, F_OUT = 1433, 16, 7
N_CORES = 8
P = 128
NPC = N_NODES // N_CORES           # 6250
NT = 49                            # node tiles per core
NPC_PAD = NT * P                   # 6272
KPAD = 1536                        # 1433 + bias row, padded to 12*128
KT = 12                            # k-chunks in L1
NPAD = 50816                       # 397*128 = 794*64 (global padded nodes)
GCH = 64                           # gather chunk (src nodes)
N_CH = NPAD // GCH                 # 794
N_CH_PAD = 800
TG = 3                             # gather tiles per chunk
CAPG = TG * P                      # 384
NG = N_CH_PAD * TG                 # 2400 gather tiles
GB = 96                            # gather tiles per DMA batch (25 batches)
WIN = 32                           # scatter window (dst nodes)
N_WIN = NPC_PAD // WIN             # 196
T1 = 10                            # scatter tiles per window
CAPS = T1 * P                      # 1280
N_S = N_WIN * T1                   # 1960 scatter tiles
SB = 160                           # scatter tiles per DMA batch

# ------------------------------------------------------------------ patches
import concourse.tile as tile_mod
from concourse.tile import TileContext
from concourse.vector_clock import ScopedClock


def _patched_drain_and_barrier(self, tick_clock, wait_clock):
    nc = self.nc
    probe = nc.sync.nop(nofuse=True, hint="drain_wait_probe")
    wait_clock.add_sem_waits(probe.ins, ScopedClock({None: tick_clock.global_clock}))
    si = probe.ins.sync_info
    waits = list(si.on_wait) if si is not None else []
    if len(waits) > 1:
        probe.ins.sync_info = mybir.SyncInfo(on_update=list(si.on_update),
                                             on_wait=waits[:1])
        for w in waits[1:]:
            extra = nc.sync.nop(nofuse=True, hint="drain_wait_spill")
            extra.ins.sync_info = mybir.SyncInfo(on_update=[], on_wait=[w])
    nc.sync.drain()
    nc.all_engine_barrier()
    assert self.sems is not None
    popped = nc._tile_sem_poison_stack.pop()
    assert popped is self._sem_poison
    nc.clear_and_free_semaphores(list(self.sems.allocated().values()))
    nc.all_engine_barrier()


tile_mod.TileContext._drain_and_barrier = _patched_drain_and_barrier

_orig_lower = tile_mod.TileContext._lower_ordered_insts


def _split_multi_waits(ordered):
    for insts in ordered.values():
        out = []
        for inst in insts:
            si = getattr(inst, "sync_info", None)
            waits = list(si.on_wait) if si is not None and si.on_wait else []
            if len(waits) > 1:
                for k, w in enumerate(waits[:-1]):
                    out.append(mybir.InstNoOp(
                        name=f"{inst.name}-wsplit{k}", engine=inst.engine,
                        bass_nofuse=True,
                        sync_info=mybir.SyncInfo(on_wait=[w], on_update=[])))
                inst.sync_info = mybir.SyncInfo(on_wait=[waits[-1]],
                                                on_update=list(si.on_update))
            out.append(inst)
        insts[:] = out


def _patched_lower(self, ordered):
    _split_multi_waits(ordered)
    return _orig_lower(self, ordered)


tile_mod.TileContext._lower_ordered_insts = _patched_lower

# ------------------------------------------------------------------ launches


def build_L1():
    nc = bass.Bass()
    xTr = nc.dram_tensor("xTr", [P, NT * KT * P], mybir.dt.bfloat16,
                         kind="ExternalInput")
    Wcp = nc.dram_tensor("Wcp", [P, KT * 48], mybir.dt.bfloat16,
                         kind="ExternalInput")
    table = nc.dram_tensor("table", [P, NT * 32], mybir.dt.bfloat16,
                           kind="ExternalOutput")
    root = nc.dram_tensor("root", [P, NT * 16], mybir.dt.float32,
                          kind="ExternalOutput")
    with TileContext(nc) as tc:
        with tc.tile_pool(name="w", bufs=1) as wpool, \
             tc.tile_pool(name="x", bufs=6) as xpool, \
             tc.tile_pool(name="o", bufs=2) as opool, \
             tc.tile_pool(name="ps", bufs=2, space="PSUM") as pspool:
            engs = [nc.sync, nc.gpsimd, nc.scalar]
            wt = wpool.tile([P, KT, 48], mybir.dt.bfloat16)
            nc.sync.dma_start(out=wt[:],
                              in_=Wcp[:].rearrange("p (a f) -> p a f", f=48))
            groups = [(g0, min(10, NT - g0)) for g0 in range(0, NT, 10)]
            qi = 0
            for g0, gn in groups:
                ps = pspool.tile([P, 10, 48], mybir.dt.float32, tag="ps")
                tb = opool.tile([P, 10, 32], mybir.dt.bfloat16, tag="tb")
                rt = opool.tile([P, 10, 16], mybir.dt.float32, tag="rt")
                for j in range(gn):
                    t = g0 + j
                    xt = xpool.tile([P, KT, P], mybir.dt.bfloat16, tag="xt")
                    engs[qi % 3].dma_start(
                        out=xt[:],
                        in_=xTr[:, t * KT * P:(t + 1) * KT * P].rearrange(
                            "p (a n) -> p a n", n=P))
                    qi += 1
                    for k in range(KT):
                        nc.tensor.matmul(out=ps[:, j, :], lhsT=xt[:, k, :],
                                         rhs=wt[:, k, :],
                                         start=(k == 0), stop=(k == KT - 1))
                nc.scalar.copy(out=tb[:, 0:gn, :], in_=ps[:, 0:gn, 0:32])
                nc.vector.tensor_copy(out=rt[:, 0:gn, :], in_=ps[:, 0:gn, 32:48])
                engs[qi % 3].dma_start(
                    out=table[:, g0 * 32:(g0 + gn) * 32],
                    in_=tb[:, 0:gn, :].rearrange("p a f -> p (a f)"))
                qi += 1
                engs[qi % 3].dma_start(
                    out=root[:, g0 * 16:(g0 + gn) * 16],
                    in_=rt[:, 0:gn, :].rearrange("p a f -> p (a f)"))
                qi += 1
    return nc


def build_gather(fdim):
    """L2 (fdim=16) / L4 (fdim=7): weighted-indicator gather.

    msg[slot] = (1-u)*h0[src] + u*h1[src] via one matmul per 128-slot tile:
    lhsT = B tile [128, 128] fp8 (rows 0:64 carry 1-u at src%64, rows 64:128
    carry u), rhs = stacked table chunk [128, fdim] bf16.
    """
    nc = bass.Bass()
    tabS = nc.dram_tensor("tabS", [P, N_CH_PAD * fdim], mybir.dt.bfloat16,
                          kind="ExternalInput")
    Bt = nc.dram_tensor("Bt", [P, NG * P], mybir.dt.float8e3,
                        kind="ExternalInput")
    msgs = nc.dram_tensor("msgs", [P, NG * fdim], mybir.dt.bfloat16,
                          kind="ExternalOutput")
    with TileContext(nc) as tc:
        with tc.tile_pool(name="tab", bufs=1) as tpool, \
             tc.tile_pool(name="b", bufs=3) as bpool, \
             tc.tile_pool(name="m", bufs=3) as mpool, \
             tc.tile_pool(name="ps", bufs=4, space="PSUM") as pspool:
            engs = [nc.sync, nc.gpsimd, nc.scalar]
            tab = tpool.tile([P, N_CH_PAD, fdim], mybir.dt.bfloat16)
            cuts = [0, 267, 534, N_CH_PAD]
            for k in range(3):
                engs[k].dma_start(
                    out=tab[:, cuts[k]:cuts[k + 1], :],
                    in_=tabS[:, cuts[k] * fdim:cuts[k + 1] * fdim].rearrange(
                        "p (a f) -> p a f", f=fdim))
            qi = 0
            for t0 in range(0, NG, GB):
                bt = bpool.tile([P, GB, P], mybir.dt.float8e3, tag="bt")
                engs[qi % 3].dma_start(
                    out=bt[:],
                    in_=Bt[:, t0 * P:(t0 + GB) * P].rearrange(
                        "p (a c) -> p a c", c=P))
                qi += 1
                mt = mpool.tile([P, GB, fdim], mybir.dt.bfloat16, tag="mt")
                for g0 in range(0, GB, 32):
                    ps = pspool.tile([P, 32, fdim], mybir.dt.float32, tag="ps")
                    for j in range(32):
                        t = t0 + g0 + j
                        nc.tensor.matmul(out=ps[:, j, :], lhsT=bt[:, g0 + j, :],
                                         rhs=tab[:, t // TG, :],
                                         start=True, stop=True)
                    nc.vector.tensor_copy(out=mt[:, g0:g0 + 32, :], in_=ps[:])
                engs[qi % 3].dma_start(
                    out=msgs[:, t0 * fdim:(t0 + GB) * fdim],
                    in_=mt[:].rearrange("p a c -> p (a c)"))
                qi += 1
    return nc


def build_segsum(fdim, layer1):
    """L3 (fdim=16, layer1) / L5 (fdim=7): windowed segment-sum + tail.

    Scatter matmuls: lhsT = pure one-hot [128 slots, 32] fp8, rhs = msg tile
    [128, fdim] bf16, accumulated T1 per window; window w lands at psum
    partitions 32*(w%4) and free slot w//4 so node n sits at [n%128, n//128].
    """
    nc = bass.Bass()
    msgs = nc.dram_tensor("msgs", [P, N_S * fdim], mybir.dt.bfloat16,
                          kind="ExternalInput")
    Sv = nc.dram_tensor("S", [P, N_S * WIN], mybir.dt.float8e3,
                        kind="ExternalInput")
    invd = nc.dram_tensor("invd", [P, NT], mybir.dt.float32,
                          kind="ExternalInput")
    root = nc.dram_tensor("root", [P, NT * fdim], mybir.dt.float32,
                          kind="ExternalInput")
    if layer1:
        Wc2 = nc.dram_tensor("Wc2", [16, 21], mybir.dt.bfloat16,
                             kind="ExternalInput")
        b2row = nc.dram_tensor("b2row", [1, 21], mybir.dt.bfloat16,
                               kind="ExternalInput")
        ones1 = nc.dram_tensor("ones1", [1, P], mybir.dt.bfloat16,
                               kind="ExternalInput")
        id128 = nc.dram_tensor("id128", [P, P], mybir.dt.bfloat16,
                               kind="ExternalInput")
        tab2 = nc.dram_tensor("tab2", [P, NT * 14], mybir.dt.bfloat16,
                              kind="ExternalOutput")
        root2v = nc.dram_tensor("root2v", [P, NT * 7], mybir.dt.float32,
                                kind="ExternalOutput")
    else:
        out = nc.dram_tensor("out", [P, NT * 7], mybir.dt.float32,
                             kind="ExternalOutput")
    # psum agg layout: fdim=16 -> two banks of 32 slots; fdim=7 -> one bank
    # of 64 slots (8 cols allocated, 7 used).
    acols = 16 if fdim == 16 else 8
    with TileContext(nc) as tc:
        with tc.tile_pool(name="sc", bufs=1) as scpool, \
             tc.tile_pool(name="m", bufs=3) as mpool, \
             tc.tile_pool(name="s", bufs=3) as spool, \
             tc.tile_pool(name="h", bufs=1) as hpool, \
             tc.tile_pool(name="tmp", bufs=1) as tmppool, \
             tc.tile_pool(name="psA", bufs=1, space="PSUM") as psApool, \
             tc.tile_pool(name="psB", bufs=1, space="PSUM") as psBpool, \
             tc.tile_pool(name="psT", bufs=2, space="PSUM") as psTpool, \
             tc.tile_pool(name="ps2", bufs=2, space="PSUM") as ps2pool:
            engs = [nc.sync, nc.gpsimd, nc.scalar]
            invt = scpool.tile([P, NT], mybir.dt.float32, tag="invt")
            nc.sync.dma_start(out=invt[:], in_=invd[:])
            roott = scpool.tile([P, NT, fdim], mybir.dt.float32, tag="roott")
            nc.scalar.dma_start(out=roott[:],
                                in_=root[:].rearrange("p (a f) -> p a f", f=fdim))
            if layer1:
                w2t = scpool.tile([16, 21], mybir.dt.bfloat16, tag="w2t")
                nc.gpsimd.dma_start(out=w2t[:], in_=Wc2[:])
                b2t = scpool.tile([1, 21], mybir.dt.bfloat16, tag="b2t")
                nc.gpsimd.dma_start(out=b2t[:], in_=b2row[:])
                onet = scpool.tile([1, P], mybir.dt.bfloat16, tag="onet")
                nc.gpsimd.dma_start(out=onet[:], in_=ones1[:])
                idt = scpool.tile([P, P], mybir.dt.bfloat16, tag="idt")
                nc.gpsimd.dma_start(out=idt[:], in_=id128[:])
            if fdim == 16:
                aggA = psApool.tile([P, 32, 16], mybir.dt.float32)
                aggB = psBpool.tile([P, 32, 16], mybir.dt.float32)
            else:
                aggA = psApool.tile([P, 64, 8], mybir.dt.float32)
                aggB = None
            # ---- streamed segment-sum
            qi = 0
            for s0 in range(0, N_S, SB):
                nb = min(SB, N_S - s0)
                mt = mpool.tile([P, SB, fdim], mybir.dt.bfloat16, tag="mt")
                engs[qi % 3].dma_start(
                    out=mt[:, 0:nb, :],
                    in_=msgs[:, s0 * fdim:(s0 + nb) * fdim].rearrange(
                        "p (a c) -> p a c", c=fdim))
                qi += 1
                st = spool.tile([P, SB, WIN], mybir.dt.float8e3, tag="st")
                engs[qi % 3].dma_start(
                    out=st[:, 0:nb, :],
                    in_=Sv[:, s0 * WIN:(s0 + nb) * WIN].rearrange(
                        "p (a c) -> p a c", c=WIN))
                qi += 1
                for j in range(nb):
                    t = s0 + j
                    w, tw = divmod(t, T1)
                    a, q = divmod(w, 4)
                    if fdim == 16:
                        dst = (aggA[32 * q:32 * q + 32, a, :] if a < 32
                               else aggB[32 * q:32 * q + 32, a - 32, :])
                    else:
                        dst = aggA[32 * q:32 * q + 32, a, 0:7]
                    nc.tensor.matmul(out=dst, lhsT=st[:, j, :],
                                     rhs=mt[:, j, 0:fdim],
                                     start=(tw == 0), stop=(tw == T1 - 1),
                                     tile_position=(0, 32 * q))
            # ---- mean (fused from psum) + root
            hpre = hpool.tile([P, NT, fdim], mybir.dt.float32, tag="hpre")
            if fdim == 16:
                nc.vector.tensor_tensor(
                    out=hpre[:, 0:32, :], in0=aggA[:],
                    in1=invt[:, 0:32].to_broadcast([P, 32, 16]),
                    op=mybir.AluOpType.mult)
                nc.vector.tensor_tensor(
                    out=hpre[:, 32:NT, :], in0=aggB[:, 0:NT - 32, :],
                    in1=invt[:, 32:NT].to_broadcast([P, NT - 32, 16]),
                    op=mybir.AluOpType.mult)
            else:
                nc.vector.tensor_tensor(
                    out=hpre[:], in0=aggA[:, 0:NT, 0:7],
                    in1=invt[:].to_broadcast([P, NT, 7]),
                    op=mybir.AluOpType.mult)
            nc.vector.tensor_add(out=hpre[:], in0=hpre[:], in1=roott[:])
            if layer1:
                # ELU
                mneg = tmppool.tile([P, NT, 16], mybir.dt.float32, tag="mn")
                nc.vector.tensor_scalar(out=mneg[:], in0=hpre[:], scalar1=0.0,
                                        scalar2=None, op0=mybir.AluOpType.min)
                emt = tmppool.tile([P, NT, 16], mybir.dt.float32, tag="em")
                nc.scalar.activation(emt[:], mneg[:],
                                     mybir.ActivationFunctionType.Exp)
                rlu = tmppool.tile([P, NT, 16], mybir.dt.float32, tag="rl")
                nc.vector.tensor_scalar(out=rlu[:], in0=hpre[:], scalar1=0.0,
                                        scalar2=None, op0=mybir.AluOpType.max)
                h1 = hpool.tile([P, NT, 16], mybir.dt.bfloat16, tag="h1")
                nc.vector.scalar_tensor_tensor(
                    out=h1[:], in0=emt[:], scalar=-1.0, in1=rlu[:],
                    op0=mybir.AluOpType.add, op1=mybir.AluOpType.add)
                # transposes: h1 [128, t, 16] -> h1T [16, t, 128]
                h1T = hpool.tile([16, NT, P], mybir.dt.bfloat16, tag="h1T")
                for t8 in range(0, NT, 8):
                    n8 = min(8, NT - t8)
                    psT = psTpool.tile([16, 8, P], mybir.dt.bfloat16, tag="psT")
                    for k in range(n8):
                        nc.tensor.transpose(out=psT[:, k, :],
                                            in_=h1[:, t8 + k, :],
                                            identity=idt[:])
                    nc.vector.tensor_copy(out=h1T[:, t8:t8 + n8, :],
                                          in_=psT[:, 0:n8, :])
                # GEMM2: out = h1 @ [W2_0|W2_1|root2] + [0|0|b2]
                t2 = hpool.tile([P, NT, 14], mybir.dt.bfloat16, tag="t2")
                r2v = hpool.tile([P, NT, 7], mybir.dt.float32, tag="r2v")
                for t24 in range(0, NT, 24):
                    n24 = min(24, NT - t24)
                    ps2 = ps2pool.tile([P, 24, 21], mybir.dt.float32, tag="ps2")
                    for k in range(n24):
                        nc.tensor.matmul(out=ps2[:, k, :], lhsT=onet[:],
                                         rhs=b2t[:], start=True, stop=False)
                        nc.tensor.matmul(out=ps2[:, k, :],
                                         lhsT=h1T[:, t24 + k, :], rhs=w2t[:],
                                         start=False, stop=True)
                    nc.scalar.copy(out=t2[:, t24:t24 + n24, :],
                                   in_=ps2[:, 0:n24, 0:14])
                    nc.vector.tensor_copy(out=r2v[:, t24:t24 + n24, :],
                                          in_=ps2[:, 0:n24, 14:21])
                nc.sync.dma_start(out=tab2[:],
                                  in_=t2[:].rearrange("p a f -> p (a f)"))
                nc.gpsimd.dma_start(out=root2v[:],
                                    in_=r2v[:].rearrange("p a f -> p (a f)"))
            else:
                # log_softmax over the 7 logits
                mx = tmppool.tile([P, NT], mybir.dt.float32, tag="mx")
                nc.vector.tensor_reduce(out=mx[:], in_=hpre[:],
                                        axis=mybir.AxisListType.X,
                                        op=mybir.AluOpType.max)
                z = tmppool.tile([P, NT, 7], mybir.dt.float32, tag="z")
                nc.vector.tensor_sub(out=z[:], in0=hpre[:],
                                     in1=mx[:].to_broadcast([P, NT, 7]))
                ez = tmppool.tile([P, NT, 7], mybir.dt.float32, tag="ez")
                nc.scalar.activation(ez[:], z[:],
                                     mybir.ActivationFunctionType.Exp)
                se = tmppool.tile([P, NT], mybir.dt.float32, tag="se")
                nc.vector.tensor_reduce(out=se[:], in_=ez[:],
                                        axis=mybir.AxisListType.X,
                                        op=mybir.AluOpType.add)
                ls = tmppool.tile([P, NT], mybir.dt.float32, tag="ls")
                nc.scalar.activation(ls[:], se[:],
                                     mybir.ActivationFunctionType.Ln)
                ot = tmppool.tile([P, NT, 7], mybir.dt.float32, tag="ot")
                nc.vector.tensor_sub(out=ot[:], in0=z[:],
                                     in1=ls[:].to_broadcast([P, NT, 7]))
                nc.sync.dma_start(out=out[:],
                                  in_=ot[:].rearrange("p a f -> p (a f)"))
    return nc


# ------------------------------------------------------------------ host prep


def _rank_within_group(group_sorted):
    n = group_sorted.shape[0]
    if n == 0:
        return np.zeros(0, dtype=np.int64)
    first = np.searchsorted(group_sorted, group_sorted, side="left")
    return np.arange(n, dtype=np.int64) - first


def plan_core(src, dst_local, u):
    E = src.shape[0]
    # gather side (src-sorted, 64-node chunks)
    og = np.argsort(src, kind="stable")
    sg = src[og]
    chunk = sg // GCH
    rank = _rank_within_group(chunk)
    assert rank.max(initial=0) < CAPG, "gather chunk overflow"
    slot = chunk * CAPG + rank         # == tile*128 + col
    slot_of_edge = np.empty(E, dtype=np.int64)
    slot_of_edge[og] = slot
    r = sg - chunk * GCH
    uo = u[og].astype(F32)
    Bt = np.zeros((P, NG * P), dtype=F8)
    Bt[r, slot] = (1.0 - uo).astype(F8)
    Bt[r + GCH, slot] = uo.astype(F8)
    # segsum side (dst-sorted, 32-node windows)
    os_ = np.argsort(dst_local, kind="stable")
    ds = dst_local[os_]
    win = ds // WIN
    rank_s = _rank_within_group(win)
    assert rank_s.max(initial=0) < CAPS, "segsum window overflow"
    pos = win * CAPS + rank_s          # == tile*128 + row
    st_ = pos // P
    sr = pos % P
    Sm = np.zeros((P, N_S * WIN), dtype=F8)
    Sm[sr, st_ * WIN + (ds - win * WIN)] = F8(1.0)
    perm = np.zeros((P, N_S), dtype=np.int64)
    perm[sr, st_] = slot_of_edge[os_]
    deg = np.bincount(dst_local, minlength=NPC).astype(F32)
    inv_pad = np.zeros(NPC_PAD, dtype=F32)
    inv_pad[:NPC] = 1.0 / np.clip(deg, 1.0, None)
    invd = np.ascontiguousarray(inv_pad.reshape(NT, P).T)
    return Bt, Sm, perm, invd


def _permute_msgs(gmsgs, perm, fdim):
    """gather msgs [P, NG*fdim] -> scatter-slot layout [P, N_S*fdim]."""
    flat = np.ascontiguousarray(
        gmsgs.reshape(P, NG, fdim).transpose(1, 0, 2)).reshape(NG * P, fdim)
    mp = flat[perm]                    # [P, N_S, fdim]
    return np.ascontiguousarray(mp).reshape(P, N_S * fdim)


def _stack_table(tglob, fdim):
    """[NPAD, 2*fdim] -> stacked gather table [P, N_CH_PAD*fdim]."""
    m = np.arange(NPAD)
    ck, ri = m // GCH, m % GCH
    tabS = np.zeros((P, N_CH_PAD, fdim), dtype=BF16)
    tabS[ri, ck] = tglob[:, 0:fdim]
    tabS[ri + GCH, ck] = tglob[:, fdim:2 * fdim]
    return np.ascontiguousarray(tabS).reshape(P, N_CH_PAD * fdim)


# ------------------------------------------------------------------ driver


_NC_CACHE = {}


def _get_nc(name, builder):
    if name not in _NC_CACHE:
        _NC_CACHE[name] = builder()
    return _NC_CACHE[name]


def _run(name, builder, in_maps):
    from concourse.bass_utils import run_bass_kernel_spmd
    import time
    nc = _get_nc(name, builder)
    t0 = time.time()
    res = run_bass_kernel_spmd(nc, in_maps, list(range(N_CORES)))
    _run.times[name] = time.time() - t0
    return res.results


_run.times = {}


def kernel(x, edge_attr, edge_index, W1, root1, b1, W2, root2, b2):
    import os
    dbg = bool(os.environ.get("KERNEL_DEBUG"))
    x = np.asarray(x, dtype=F32)
    u = np.asarray(edge_attr, dtype=F32).reshape(-1)
    ei = np.asarray(edge_index, dtype=np.int64)
    src_all, dst_all = ei[0], ei[1]

    # --- shard edges by dst owner core
    owner = dst_all // NPC
    plans = []
    for c in range(N_CORES):
        m = owner == c
        plans.append(plan_core(src_all[m], dst_all[m] - c * NPC, u[m]))

    # --- L1: GEMM (x @ [W1_0|W1_1|root1], bias row for root part)
    Wc = np.zeros((KPAD, 48), dtype=F32)
    Wc[:F_IN, 0:16] = np.asarray(W1[0], dtype=F32)
    Wc[:F_IN, 16:32] = np.asarray(W1[1], dtype=F32)
    Wc[:F_IN, 32:48] = np.asarray(root1, dtype=F32)
    Wc[F_IN, 32:48] = np.asarray(b1, dtype=F32)
    Wcp = np.ascontiguousarray(
        Wc.reshape(KT, P, 48).transpose(1, 0, 2)).reshape(P, KT * 48).astype(BF16)
    in1 = []
    for c in range(N_CORES):
        xf = np.zeros((NPC_PAD, KPAD), dtype=BF16)
        xf[:NPC, :F_IN] = x[c * NPC:(c + 1) * NPC].astype(BF16)
        xf[:NPC, F_IN] = BF16(1.0)
        xTr = np.ascontiguousarray(
            xf.reshape(NT, P, KT, P).transpose(3, 0, 2, 1)).reshape(P, NT * KT * P)
        in1.append({"xTr": xTr, "Wcp": Wcp})
    r1 = _run("L1", build_L1, in1)
    tglob1 = np.zeros((NPAD, 32), dtype=BF16)
    roots = []
    for c in range(N_CORES):
        tl = r1[c]["table"].reshape(P, NT, 32).transpose(1, 0, 2).reshape(NPC_PAD, 32)
        tglob1[c * NPC:(c + 1) * NPC] = tl[:NPC]
        roots.append(np.ascontiguousarray(r1[c]["root"]))
    if dbg:
        xfull = np.zeros((N_NODES, KPAD), dtype=F32)
        xfull[:, :F_IN] = x
        xfull[:, F_IN] = 1.0
        Hexp = xfull @ Wc
        got = tglob1[:N_NODES].astype(F32)
        print("L1 table relerr:",
              np.abs(got - Hexp[:, 0:32]).max() / np.abs(Hexp[:, 0:32]).max())
        r0 = roots[0].reshape(P, NT, 16).transpose(1, 0, 2).reshape(NPC_PAD, 16)
        print("L1 root relerr:",
              np.abs(r0[:NPC] - Hexp[:NPC, 32:48]).max() / np.abs(Hexp[:, 32:48]).max())

    # --- L2: weighted gather layer 1
    tabS1 = _stack_table(tglob1, 16)
    in2 = [{"tabS": tabS1, "Bt": plans[c][0]} for c in range(N_CORES)]
    r2 = _run("L2", lambda: build_gather(16), in2)
    if dbg:
        c = 0
        m = owner == c
        s0, u0 = src_all[m], u[m]
        og = np.argsort(s0, kind="stable")
        sl = plans[c][2]  # perm (not needed); recompute slots
        sg = s0[og]
        ch = sg // GCH
        rk = _rank_within_group(ch)
        slots = ch * CAPG + rk
        tabf = tglob1.astype(F32)
        exp_msg = ((1 - u0[og])[:, None] * tabf[sg, 0:16]
                   + u0[og][:, None] * tabf[sg, 16:32])
        gm = r2[c]["msgs"].reshape(P, NG, 16)
        got = gm[slots % P, slots // P].astype(F32)
        print("L2 msg relerr:",
              np.abs(got - exp_msg).max() / np.abs(exp_msg).max())

    # --- L3: segsum + mean + root + ELU + GEMM2
    Wc2 = np.zeros((16, 21), dtype=BF16)
    Wc2[:, 0:7] = np.asarray(W2[0], dtype=BF16)
    Wc2[:, 7:14] = np.asarray(W2[1], dtype=BF16)
    Wc2[:, 14:21] = np.asarray(root2, dtype=BF16)
    b2row = np.zeros((1, 21), dtype=BF16)
    b2row[0, 14:21] = np.asarray(b2, dtype=BF16)
    ones1 = np.ones((1, P), dtype=BF16)
    id128 = np.eye(P, dtype=BF16)
    in3 = []
    for c in range(N_CORES):
        in3.append({"msgs": _permute_msgs(r2[c]["msgs"], plans[c][2], 16),
                    "S": plans[c][1], "invd": plans[c][3], "root": roots[c],
                    "Wc2": Wc2, "b2row": b2row, "ones1": ones1,
                    "id128": id128})
    r3 = _run("L3", lambda: build_segsum(16, True), in3)
    tglob2 = np.zeros((NPAD, 14), dtype=BF16)
    roots2 = []
    for c in range(N_CORES):
        tl = r3[c]["tab2"].reshape(P, NT, 14).transpose(1, 0, 2).reshape(NPC_PAD, 14)
        tglob2[c * NPC:(c + 1) * NPC] = tl[:NPC]
        roots2.append(np.ascontiguousarray(r3[c]["root2v"]))

    # --- L4: weighted gather layer 2
    tabS2 = _stack_table(tglob2, 7)
    in4 = [{"tabS": tabS2, "Bt": plans[c][0]} for c in range(N_CORES)]
    r4 = _run("L4", lambda: build_gather(7), in4)

    # --- L5: segsum + mean + root2 + log_softmax
    in5 = []
    for c in range(N_CORES):
        in5.append({"msgs": _permute_msgs(r4[c]["msgs"], plans[c][2], 7),
                    "S": plans[c][1], "invd": plans[c][3],
                    "root": roots2[c]})
    r5 = _run("L5", lambda: build_segsum(7, False), in5)

    out = np.zeros((N_NODES, F_OUT), dtype=F32)
    for c in range(N_CORES):
        ol = r5[c]["out"].reshape(P, NT, 7).transpose(1, 0, 2).reshape(NPC_PAD, 7)
        out[c * NPC:(c + 1) * NPC] = ol[:NPC]
    return out


# revision 11
# speedup vs baseline: 6.8138x; 1.2048x over previous
"""SplineConv 2-layer GNN (nn_Net_23587960389976) on 8 trn2 NeuronCores.

Structure: 5 SPMD bass launches. All value arithmetic runs on device; the
host only shards, permutes by precomputed indices, and concatenates.

  L1: H = x_shard @ [W1_0|W1_1|root1+b1row]  -> table shard (bf16) + root (f32)
  L2: per-edge basis-weighted gather via fp8 weighted-indicator matmuls:
      64-src-node chunks, lhsT column s holds (1-u_e) at row src%64 and
      u_e at row 64+src%64, rhs = [h0;h1] stacked table chunk -> msg directly.
  L3: windowed segment-sum (32-node dst windows, pure fp8 one-hot scatter
      matmuls packed 4 windows/psum partition group) + mean + root + ELU
      + GEMM2 (PE transposes + matmul, bias via K=1 ones matmul) -> table2/root2
  L4: weighted gather layer 2 (same B matrices, 7-col table)
  L5: segment-sum + mean + root2 + log_softmax

Cost-model-aware choices: matmuls are charged only out-free-size cycles, so
all gather/scatter work rides the PE; DMA is charged per-partition bytes on
the issuing engine queue, so bulk traffic is fp8 and round-robined across the
three DMA-capable queues (SP/sync, Pool/gpsimd, Act/scalar); per-instruction
vector/scalar engine overhead (~60-185ns) is amortized by batching all
DVE/Act ops over >=512-element tiles.

Per-core edge schedule is SPMD-uniform with fixed capacities:
  gather: 3 tiles of 128 slots per 64-src-chunk (384 >= max 320 on seed-0)
  segsum: 10 tiles of 128 slots per 32-dst-window (1280 >= max 1115)
"""
import sys

sys.path.insert(0, "/opt/trn_rl_repo")

import numpy as np
import ml_dtypes

import concourse.bass as bass
import concourse.mybir as mybir

BF16 = ml_dtypes.bfloat16
F8 = ml_dtypes.float8_e3m4
F32 = np.float32

N_NODES = 50000
N_EDGES = 1600000
F_IN, F_HID, F_OUT = 1433, 16, 7
N_CORES = 8
P = 128
NPC = N_NODES // N_CORES           # 6250
NT = 49                            # node tiles per core
NPC_PAD = NT * P                   # 6272
KPAD = 1536                        # 1433 + bias row, padded to 12*128
KT = 12                            # k-chunks in L1
NPAD = 50816                       # 397*128 = 794*64 (global padded nodes)
GCH = 64                           # gather chunk (src nodes)
N_CH = NPAD // GCH                 # 794
N_CH_PAD = 800
TG = 3                             # gather tiles per chunk
CAPG = TG * P                      # 384
NG = N_CH_PAD * TG                 # 2400 gather tiles
GB = 96                            # gather tiles per DMA batch (25 batches)
WIN = 32                           # scatter window (dst nodes)
N_WIN = NPC_PAD // WIN             # 196
T1 = 10                            # scatter tiles per window
CAPS = T1 * P                      # 1280
N_S = N_WIN * T1                   # 1960 scatter tiles
SB = 160                           # scatter tiles per DMA batch

# ------------------------------------------------------------------ patches
import concourse.tile as tile_mod
from concourse.tile import TileContext
from concourse.vector_clock import ScopedClock


def _patched_drain_and_barrier(self, tick_clock, wait_clock):
    nc = self.nc
    probe = nc.sync.nop(nofuse=True, hint="drain_wait_probe")
    wait_clock.add_sem_waits(probe.ins, ScopedClock({None: tick_clock.global_clock}))
    si = probe.ins.sync_info
    waits = list(si.on_wait) if si is not None else []
    if len(waits) > 1:
        probe.ins.sync_info = mybir.SyncInfo(on_update=list(si.on_update),
                                             on_wait=waits[:1])
        for w in waits[1:]:
            extra = nc.sync.nop(nofuse=True, hint="drain_wait_spill")
            extra.ins.sync_info = mybir.SyncInfo(on_update=[], on_wait=[w])
    nc.sync.drain()
    nc.all_engine_barrier()
    assert self.sems is not None
    popped = nc._tile_sem_poison_stack.pop()
    assert popped is self._sem_poison
    nc.clear_and_free_semaphores(list(self.sems.allocated().values()))
    nc.all_engine_barrier()


tile_mod.TileContext._drain_and_barrier = _patched_drain_and_barrier

_orig_lower = tile_mod.TileContext._lower_ordered_insts


def _split_multi_waits(ordered):
    for insts in ordered.values():
        out = []
        for inst in insts:
            si = getattr(inst, "sync_info", None)
            waits = list(si.on_wait) if si is not None and si.on_wait else []
            if len(waits) > 1:
                for k, w in enumerate(waits[:-1]):
                    out.append(mybir.InstNoOp(
                        name=f"{inst.name}-wsplit{k}", engine=inst.engine,
                        bass_nofuse=True,
                        sync_info=mybir.SyncInfo(on_wait=[w], on_update=[])))
                inst.sync_info = mybir.SyncInfo(on_wait=[waits[-1]],
                                                on_update=list(si.on_update))
            out.append(inst)
        insts[:] = out


def _patched_lower(self, ordered):
    _split_multi_waits(ordered)
    return _orig_lower(self, ordered)


tile_mod.TileContext._lower_ordered_insts = _patched_lower

# ------------------------------------------------------------------ launches


def build_L1():
    nc = bass.Bass()
    xTr = nc.dram_tensor("xTr", [P, NT * KT * P], mybir.dt.bfloat16,
                         kind="ExternalInput")
    Wcp = nc.dram_tensor("Wcp", [P, KT * 48], mybir.dt.bfloat16,
                         kind="ExternalInput")
    table = nc.dram_tensor("table", [P, NT * 32], mybir.dt.bfloat16,
                           kind="ExternalOutput")
    root = nc.dram_tensor("root", [P, NT * 16], mybir.dt.float32,
                          kind="ExternalOutput")
    with TileContext(nc) as tc:
        with tc.tile_pool(name="w", bufs=1) as wpool, \
             tc.tile_pool(name="x", bufs=12) as xpool, \
             tc.tile_pool(name="o", bufs=2) as opool, \
             tc.tile_pool(name="ps", bufs=6, space="PSUM") as pspool:
            engs = [nc.sync, nc.gpsimd, nc.scalar]
            wt = wpool.tile([P, KT, 48], mybir.dt.bfloat16)
            nc.sync.dma_start(out=wt[:],
                              in_=Wcp[:].rearrange("p (a f) -> p a f", f=48))
            XB = 2                  # node tiles per xt DMA
            groups = [(g0, min(5, NT - g0)) for g0 in range(0, NT, 5)]
            qi = 0
            for g0, gn in groups:
                ps = pspool.tile([P, 5, 48], mybir.dt.float32, tag="ps")
                tb = opool.tile([P, 5, 32], mybir.dt.bfloat16, tag="tb")
                rt = opool.tile([P, 5, 16], mybir.dt.float32, tag="rt")
                for j0 in range(0, gn, XB):
                    nx = min(XB, gn - j0)
                    t = g0 + j0
                    xt = xpool.tile([P, XB, KT, P], mybir.dt.bfloat16, tag="xt")
                    engs[qi % 3].dma_start(
                        out=xt[:, 0:nx, :, :],
                        in_=xTr[:, t * KT * P:(t + nx) * KT * P].rearrange(
                            "p (b a n) -> p b a n", a=KT, n=P))
                    qi += 1
                    for i in range(nx):
                        for k in range(KT):
                            nc.tensor.matmul(out=ps[:, j0 + i, :],
                                             lhsT=xt[:, i, k, :],
                                             rhs=wt[:, k, :],
                                             start=(k == 0), stop=(k == KT - 1))
                nc.scalar.copy(out=tb[:, 0:gn, :], in_=ps[:, 0:gn, 0:32])
                nc.vector.tensor_copy(out=rt[:, 0:gn, :], in_=ps[:, 0:gn, 32:48])
                engs[qi % 3].dma_start(
                    out=table[:, g0 * 32:(g0 + gn) * 32],
                    in_=tb[:, 0:gn, :].rearrange("p a f -> p (a f)"))
                qi += 1
                engs[qi % 3].dma_start(
                    out=root[:, g0 * 16:(g0 + gn) * 16],
                    in_=rt[:, 0:gn, :].rearrange("p a f -> p (a f)"))
                qi += 1
    return nc


def build_gather(fdim):
    """L2 (fdim=16) / L4 (fdim=7): weighted-indicator gather.

    msg[slot] = (1-u)*h0[src] + u*h1[src] via one matmul per 128-slot tile:
    lhsT = B tile [128, 128] fp8 (rows 0:64 carry 1-u at src%64, rows 64:128
    carry u), rhs = stacked table chunk [128, fdim] bf16.
    """
    nc = bass.Bass()
    tabS = nc.dram_tensor("tabS", [P, N_CH_PAD * fdim], mybir.dt.bfloat16,
                          kind="ExternalInput")
    Bt = nc.dram_tensor("Bt", [P, NG * P], mybir.dt.float8e3,
                        kind="ExternalInput")
    msgs = nc.dram_tensor("msgs", [P, NG * fdim], mybir.dt.bfloat16,
                          kind="ExternalOutput")
    with TileContext(nc) as tc:
        with tc.tile_pool(name="tab", bufs=1) as tpool, \
             tc.tile_pool(name="b", bufs=6) as bpool, \
             tc.tile_pool(name="m", bufs=6) as mpool, \
             tc.tile_pool(name="ps", bufs=8, space="PSUM") as pspool:
            engs = [nc.sync, nc.gpsimd, nc.scalar]
            tab = tpool.tile([P, N_CH_PAD, fdim], mybir.dt.bfloat16)
            cuts = [0, 267, 534, N_CH_PAD]
            for k in range(3):
                engs[k].dma_start(
                    out=tab[:, cuts[k]:cuts[k + 1], :],
                    in_=tabS[:, cuts[k] * fdim:cuts[k + 1] * fdim].rearrange(
                        "p (a f) -> p a f", f=fdim))
            qi = 0
            for t0 in range(0, NG, GB):
                bt = bpool.tile([P, GB, P], mybir.dt.float8e3, tag="bt")
                engs[qi % 3].dma_start(
                    out=bt[:],
                    in_=Bt[:, t0 * P:(t0 + GB) * P].rearrange(
                        "p (a c) -> p a c", c=P))
                qi += 1
                mt = mpool.tile([P, GB, fdim], mybir.dt.bfloat16, tag="mt")
                for g0 in range(0, GB, 32):
                    ps = pspool.tile([P, 32, fdim], mybir.dt.float32, tag="ps")
                    for j in range(32):
                        t = t0 + g0 + j
                        nc.tensor.matmul(out=ps[:, j, :], lhsT=bt[:, g0 + j, :],
                                         rhs=tab[:, t // TG, :],
                                         start=True, stop=True)
                    nc.vector.tensor_copy(out=mt[:, g0:g0 + 32, :], in_=ps[:])
                engs[qi % 3].dma_start(
                    out=msgs[:, t0 * fdim:(t0 + GB) * fdim],
                    in_=mt[:].rearrange("p a c -> p (a c)"))
                qi += 1
    return nc


def build_segsum(fdim, layer1):
    """L3 (fdim=16, layer1) / L5 (fdim=7): windowed segment-sum + tail.

    Scatter matmuls: lhsT = pure one-hot [128 slots, 32] fp8, rhs = msg tile
    [128, fdim] bf16, accumulated T1 per window; window w lands at psum
    partitions 32*(w%4) and free slot w//4 so node n sits at [n%128, n//128].
    """
    nc = bass.Bass()
    msgs = nc.dram_tensor("msgs", [P, N_S * fdim], mybir.dt.bfloat16,
                          kind="ExternalInput")
    Sv = nc.dram_tensor("S", [P, N_S * WIN], mybir.dt.float8e3,
                        kind="ExternalInput")
    invd = nc.dram_tensor("invd", [P, NT], mybir.dt.float32,
                          kind="ExternalInput")
    root = nc.dram_tensor("root", [P, NT * fdim], mybir.dt.float32,
                          kind="ExternalInput")
    if layer1:
        Wc2 = nc.dram_tensor("Wc2", [16, 21], mybir.dt.bfloat16,
                             kind="ExternalInput")
        b2row = nc.dram_tensor("b2row", [1, 21], mybir.dt.bfloat16,
                               kind="ExternalInput")
        ones1 = nc.dram_tensor("ones1", [1, P], mybir.dt.bfloat16,
                               kind="ExternalInput")
        id128 = nc.dram_tensor("id128", [P, P], mybir.dt.bfloat16,
                               kind="ExternalInput")
        tab2 = nc.dram_tensor("tab2", [P, NT * 14], mybir.dt.bfloat16,
                              kind="ExternalOutput")
        root2v = nc.dram_tensor("root2v", [P, NT * 7], mybir.dt.float32,
                                kind="ExternalOutput")
    else:
        out = nc.dram_tensor("out", [P, NT * 7], mybir.dt.float32,
                             kind="ExternalOutput")
    # psum agg: one bank per half (slots 0..31 = windows 0..127, slots
    # 32..48 = windows 128..195) so each half's tail overlaps the other
    # half's scatter stream.
    acols = 16 if fdim == 16 else 8
    halves = [(0, 32), (32, NT)]
    with TileContext(nc) as tc:
        with tc.tile_pool(name="sc", bufs=1) as scpool, \
             tc.tile_pool(name="m", bufs=3) as mpool, \
             tc.tile_pool(name="s", bufs=3) as spool, \
             tc.tile_pool(name="h", bufs=1) as hpool, \
             tc.tile_pool(name="tmp", bufs=1) as tmppool, \
             tc.tile_pool(name="psA", bufs=1, space="PSUM") as psApool, \
             tc.tile_pool(name="psB", bufs=1, space="PSUM") as psBpool, \
             tc.tile_pool(name="psT", bufs=2, space="PSUM") as psTpool, \
             tc.tile_pool(name="ps2", bufs=2, space="PSUM") as ps2pool:
            engs = [nc.sync, nc.gpsimd, nc.scalar]
            invt = scpool.tile([P, NT], mybir.dt.float32, tag="invt")
            nc.sync.dma_start(out=invt[:], in_=invd[:])
            roott = scpool.tile([P, NT, fdim], mybir.dt.float32, tag="roott")
            nc.scalar.dma_start(out=roott[:],
                                in_=root[:].rearrange("p (a f) -> p a f", f=fdim))
            if layer1:
                w2t = scpool.tile([16, 21], mybir.dt.bfloat16, tag="w2t")
                nc.gpsimd.dma_start(out=w2t[:], in_=Wc2[:])
                b2t = scpool.tile([1, 21], mybir.dt.bfloat16, tag="b2t")
                nc.gpsimd.dma_start(out=b2t[:], in_=b2row[:])
                onet = scpool.tile([1, P], mybir.dt.bfloat16, tag="onet")
                nc.gpsimd.dma_start(out=onet[:], in_=ones1[:])
                idt = scpool.tile([P, P], mybir.dt.bfloat16, tag="idt")
                nc.gpsimd.dma_start(out=idt[:], in_=id128[:])
            aggs = [psApool.tile([P, 32, acols], mybir.dt.float32,
                                 name="aggA"),
                    psBpool.tile([P, 32, acols], mybir.dt.float32,
                                 name="aggB")]

            def tail(h):
                lo, hi = halves[h]
                ns = hi - lo
                agg = aggs[h]
                hpre = hpool.tile([P, ns, fdim], mybir.dt.float32,
                                  tag=f"hpre{h}")
                nc.vector.tensor_tensor(
                    out=hpre[:], in0=agg[:, 0:ns, 0:fdim],
                    in1=invt[:, lo:hi].to_broadcast([P, ns, fdim]),
                    op=mybir.AluOpType.mult)
                nc.vector.tensor_add(out=hpre[:], in0=hpre[:],
                                     in1=roott[:, lo:hi, :])
                if layer1:
                    # ELU
                    mneg = tmppool.tile([P, ns, 16], mybir.dt.float32,
                                        tag=f"mn{h}")
                    nc.vector.tensor_scalar(out=mneg[:], in0=hpre[:],
                                            scalar1=0.0, scalar2=None,
                                            op0=mybir.AluOpType.min)
                    emt = tmppool.tile([P, ns, 16], mybir.dt.float32,
                                       tag=f"em{h}")
                    nc.scalar.activation(emt[:], mneg[:],
                                         mybir.ActivationFunctionType.Exp)
                    rlu = tmppool.tile([P, ns, 16], mybir.dt.float32,
                                       tag=f"rl{h}")
                    nc.vector.tensor_scalar(out=rlu[:], in0=hpre[:],
                                            scalar1=0.0, scalar2=None,
                                            op0=mybir.AluOpType.max)
                    h1 = hpool.tile([P, ns, 16], mybir.dt.bfloat16,
                                    tag=f"h1{h}")
                    nc.vector.scalar_tensor_tensor(
                        out=h1[:], in0=emt[:], scalar=-1.0, in1=rlu[:],
                        op0=mybir.AluOpType.add, op1=mybir.AluOpType.add)
                    # transposes: h1 [128, t, 16] -> h1T [16, t, 128]
                    h1T = hpool.tile([16, ns, P], mybir.dt.bfloat16,
                                     tag=f"h1T{h}")
                    for t8 in range(0, ns, 8):
                        n8 = min(8, ns - t8)
                        psT = psTpool.tile([16, 8, P], mybir.dt.bfloat16,
                                           tag="psT")
                        for k in range(n8):
                            nc.tensor.transpose(out=psT[:, k, :],
                                                in_=h1[:, t8 + k, :],
                                                identity=idt[:])
                        nc.vector.tensor_copy(out=h1T[:, t8:t8 + n8, :],
                                              in_=psT[:, 0:n8, :])
                    # GEMM2: out = h1 @ [W2_0|W2_1|root2] + [0|0|b2]
                    t2 = hpool.tile([P, ns, 14], mybir.dt.bfloat16,
                                    tag=f"t2{h}")
                    r2v = hpool.tile([P, ns, 7], mybir.dt.float32,
                                     tag=f"r2v{h}")
                    for t24 in range(0, ns, 24):
                        n24 = min(24, ns - t24)
                        ps2 = ps2pool.tile([P, 24, 21], mybir.dt.float32,
                                           tag="ps2")
                        for k in range(n24):
                            nc.tensor.matmul(out=ps2[:, k, :], lhsT=onet[:],
                                             rhs=b2t[:], start=True,
                                             stop=False)
                            nc.tensor.matmul(out=ps2[:, k, :],
                                             lhsT=h1T[:, t24 + k, :],
                                             rhs=w2t[:], start=False,
                                             stop=True)
                        nc.scalar.copy(out=t2[:, t24:t24 + n24, :],
                                       in_=ps2[:, 0:n24, 0:14])
                        nc.vector.tensor_copy(out=r2v[:, t24:t24 + n24, :],
                                              in_=ps2[:, 0:n24, 14:21])
                    nc.sync.dma_start(
                        out=tab2[:, lo * 14:hi * 14],
                        in_=t2[:].rearrange("p a f -> p (a f)"))
                    nc.gpsimd.dma_start(
                        out=root2v[:, lo * 7:hi * 7],
                        in_=r2v[:].rearrange("p a f -> p (a f)"))
                else:
                    # log_softmax over the 7 logits
                    mx = tmppool.tile([P, ns], mybir.dt.float32, tag=f"mx{h}")
                    nc.vector.tensor_reduce(out=mx[:], in_=hpre[:],
                                            axis=mybir.AxisListType.X,
                                            op=mybir.AluOpType.max)
                    z = tmppool.tile([P, ns, 7], mybir.dt.float32,
                                     tag=f"z{h}")
                    nc.vector.tensor_sub(out=z[:], in0=hpre[:],
                                         in1=mx[:].to_broadcast([P, ns, 7]))
                    ez = tmppool.tile([P, ns, 7], mybir.dt.float32,
                                      tag=f"ez{h}")
                    nc.scalar.activation(ez[:], z[:],
                                         mybir.ActivationFunctionType.Exp)
                    se = tmppool.tile([P, ns], mybir.dt.float32, tag=f"se{h}")
                    nc.vector.tensor_reduce(out=se[:], in_=ez[:],
                                            axis=mybir.AxisListType.X,
                                            op=mybir.AluOpType.add)
                    ls = tmppool.tile([P, ns], mybir.dt.float32, tag=f"ls{h}")
                    nc.scalar.activation(ls[:], se[:],
                                         mybir.ActivationFunctionType.Ln)
                    ot = tmppool.tile([P, ns, 7], mybir.dt.float32,
                                      tag=f"ot{h}")
                    nc.vector.tensor_sub(out=ot[:], in0=z[:],
                                         in1=ls[:].to_broadcast([P, ns, 7]))
                    nc.sync.dma_start(
                        out=out[:, lo * 7:hi * 7],
                        in_=ot[:].rearrange("p a f -> p (a f)"))

            # ---- streamed segment-sum, half-A tail issued mid-stream
            qi = 0
            half_a_done = 32 * 4 * T1    # first tile index owned by half B
            for s0 in range(0, N_S, SB):
                nb = min(SB, N_S - s0)
                mt = mpool.tile([P, SB, fdim], mybir.dt.bfloat16, tag="mt")
                engs[qi % 3].dma_start(
                    out=mt[:, 0:nb, :],
                    in_=msgs[:, s0 * fdim:(s0 + nb) * fdim].rearrange(
                        "p (a c) -> p a c", c=fdim))
                qi += 1
                st = spool.tile([P, SB, WIN], mybir.dt.float8e3, tag="st")
                engs[qi % 3].dma_start(
                    out=st[:, 0:nb, :],
                    in_=Sv[:, s0 * WIN:(s0 + nb) * WIN].rearrange(
                        "p (a c) -> p a c", c=WIN))
                qi += 1
                for j in range(nb):
                    t = s0 + j
                    w, tw = divmod(t, T1)
                    a, q = divmod(w, 4)
                    agg = aggs[0] if a < 32 else aggs[1]
                    dst = agg[32 * q:32 * q + 32, a % 32, 0:fdim]
                    nc.tensor.matmul(out=dst, lhsT=st[:, j, :],
                                     rhs=mt[:, j, 0:fdim],
                                     start=(tw == 0), stop=(tw == T1 - 1),
                                     tile_position=(0, 32 * q))
                if s0 < half_a_done <= s0 + nb:
                    tail(0)
            tail(1)
    return nc


# ------------------------------------------------------------------ host prep


def _rank_within_group(group_sorted):
    n = group_sorted.shape[0]
    if n == 0:
        return np.zeros(0, dtype=np.int64)
    first = np.searchsorted(group_sorted, group_sorted, side="left")
    return np.arange(n, dtype=np.int64) - first


def plan_core(src, dst_local, u):
    E = src.shape[0]
    # gather side (src-sorted, 64-node chunks)
    og = np.argsort(src, kind="stable")
    sg = src[og]
    chunk = sg // GCH
    rank = _rank_within_group(chunk)
    assert rank.max(initial=0) < CAPG, "gather chunk overflow"
    slot = chunk * CAPG + rank         # == tile*128 + col
    slot_of_edge = np.empty(E, dtype=np.int64)
    slot_of_edge[og] = slot
    r = sg - chunk * GCH
    uo = u[og].astype(F32)
    Bt = np.zeros((P, NG * P), dtype=F8)
    Bt[r, slot] = (1.0 - uo).astype(F8)
    Bt[r + GCH, slot] = uo.astype(F8)
    # segsum side (dst-sorted, 32-node windows)
    os_ = np.argsort(dst_local, kind="stable")
    ds = dst_local[os_]
    win = ds // WIN
    rank_s = _rank_within_group(win)
    assert rank_s.max(initial=0) < CAPS, "segsum window overflow"
    pos = win * CAPS + rank_s          # == tile*128 + row
    st_ = pos // P
    sr = pos % P
    Sm = np.zeros((P, N_S * WIN), dtype=F8)
    Sm[sr, st_ * WIN + (ds - win * WIN)] = F8(1.0)
    perm = np.zeros((P, N_S), dtype=np.int64)
    perm[sr, st_] = slot_of_edge[os_]
    deg = np.bincount(dst_local, minlength=NPC).astype(F32)
    inv_pad = np.zeros(NPC_PAD, dtype=F32)
    inv_pad[:NPC] = 1.0 / np.clip(deg, 1.0, None)
    invd = np.ascontiguousarray(inv_pad.reshape(NT, P).T)
    return Bt, Sm, perm, invd


def _permute_msgs(gmsgs, perm, fdim):
    """gather msgs [P, NG*fdim] -> scatter-slot layout [P, N_S*fdim]."""
    flat = np.ascontiguousarray(
        gmsgs.reshape(P, NG, fdim).transpose(1, 0, 2)).reshape(NG * P, fdim)
    mp = flat[perm]                    # [P, N_S, fdim]
    return np.ascontiguousarray(mp).reshape(P, N_S * fdim)


def _stack_table(tglob, fdim):
    """[NPAD, 2*fdim] -> stacked gather table [P, N_CH_PAD*fdim]."""
    m = np.arange(NPAD)
    ck, ri = m // GCH, m % GCH
    tabS = np.zeros((P, N_CH_PAD, fdim), dtype=BF16)
    tabS[ri, ck] = tglob[:, 0:fdim]
    tabS[ri + GCH, ck] = tglob[:, fdim:2 * fdim]
    return np.ascontiguousarray(tabS).reshape(P, N_CH_PAD * fdim)


# ------------------------------------------------------------------ driver


_NC_CACHE = {}


def _get_nc(name, builder):
    if name not in _NC_CACHE:
        _NC_CACHE[name] = builder()
    return _NC_CACHE[name]


def _run(name, builder, in_maps):
    from concourse.bass_utils import run_bass_kernel_spmd
    import time
    nc = _get_nc(name, builder)
    t0 = time.time()
    res = run_bass_kernel_spmd(nc, in_maps, list(range(N_CORES)))
    _run.times[name] = time.time() - t0
    return res.results


_run.times = {}


def kernel(x, edge_attr, edge_index, W1, root1, b1, W2, root2, b2):
    import os
    dbg = bool(os.environ.get("KERNEL_DEBUG"))
    x = np.asarray(x, dtype=F32)
    u = np.asarray(edge_attr, dtype=F32).reshape(-1)
    ei = np.asarray(edge_index, dtype=np.int64)
    src_all, dst_all = ei[0], ei[1]

    # --- shard edges by dst owner core
    owner = dst_all // NPC
    plans = []
    for c in range(N_CORES):
        m = owner == c
        plans.append(plan_core(src_all[m], dst_all[m] - c * NPC, u[m]))

    # --- L1: GEMM (x @ [W1_0|W1_1|root1], bias row for root part)
    Wc = np.zeros((KPAD, 48), dtype=F32)
    Wc[:F_IN, 0:16] = np.asarray(W1[0], dtype=F32)
    Wc[:F_IN, 16:32] = np.asarray(W1[1], dtype=F32)
    Wc[:F_IN, 32:48] = np.asarray(root1, dtype=F32)
    Wc[F_IN, 32:48] = np.asarray(b1, dtype=F32)
    Wcp = np.ascontiguousarray(
        Wc.reshape(KT, P, 48).transpose(1, 0, 2)).reshape(P, KT * 48).astype(BF16)
    in1 = []
    for c in range(N_CORES):
        xf = np.zeros((NPC_PAD, KPAD), dtype=BF16)
        xf[:NPC, :F_IN] = x[c * NPC:(c + 1) * NPC].astype(BF16)
        xf[:NPC, F_IN] = BF16(1.0)
        xTr = np.ascontiguousarray(
            xf.reshape(NT, P, KT, P).transpose(3, 0, 2, 1)).reshape(P, NT * KT * P)
        in1.append({"xTr": xTr, "Wcp": Wcp})
    r1 = _run("L1", build_L1, in1)
    tglob1 = np.zeros((NPAD, 32), dtype=BF16)
    roots = []
    for c in range(N_CORES):
        tl = r1[c]["table"].reshape(P, NT, 32).transpose(1, 0, 2).reshape(NPC_PAD, 32)
        tglob1[c * NPC:(c + 1) * NPC] = tl[:NPC]
        roots.append(np.ascontiguousarray(r1[c]["root"]))
    if dbg:
        xfull = np.zeros((N_NODES, KPAD), dtype=F32)
        xfull[:, :F_IN] = x
        xfull[:, F_IN] = 1.0
        Hexp = xfull @ Wc
        got = tglob1[:N_NODES].astype(F32)
        print("L1 table relerr:",
              np.abs(got - Hexp[:, 0:32]).max() / np.abs(Hexp[:, 0:32]).max())
        r0 = roots[0].reshape(P, NT, 16).transpose(1, 0, 2).reshape(NPC_PAD, 16)
        print("L1 root relerr:",
              np.abs(r0[:NPC] - Hexp[:NPC, 32:48]).max() / np.abs(Hexp[:, 32:48]).max())

    # --- L2: weighted gather layer 1
    tabS1 = _stack_table(tglob1, 16)
    in2 = [{"tabS": tabS1, "Bt": plans[c][0]} for c in range(N_CORES)]
    r2 = _run("L2", lambda: build_gather(16), in2)
    if dbg:
        c = 0
        m = owner == c
        s0, u0 = src_all[m], u[m]
        og = np.argsort(s0, kind="stable")
        sl = plans[c][2]  # perm (not needed); recompute slots
        sg = s0[og]
        ch = sg // GCH
        rk = _rank_within_group(ch)
        slots = ch * CAPG + rk
        tabf = tglob1.astype(F32)
        exp_msg = ((1 - u0[og])[:, None] * tabf[sg, 0:16]
                   + u0[og][:, None] * tabf[sg, 16:32])
        gm = r2[c]["msgs"].reshape(P, NG, 16)
        got = gm[slots % P, slots // P].astype(F32)
        print("L2 msg relerr:",
              np.abs(got - exp_msg).max() / np.abs(exp_msg).max())

    # --- L3: segsum + mean + root + ELU + GEMM2
    Wc2 = np.zeros((16, 21), dtype=BF16)
    Wc2[:, 0:7] = np.asarray(W2[0], dtype=BF16)
    Wc2[:, 7:14] = np.asarray(W2[1], dtype=BF16)
    Wc2[:, 14:21] = np.asarray(root2, dtype=BF16)
    b2row = np.zeros((1, 21), dtype=BF16)
    b2row[0, 14:21] = np.asarray(b2, dtype=BF16)
    ones1 = np.ones((1, P), dtype=BF16)
    id128 = np.eye(P, dtype=BF16)
    in3 = []
    for c in range(N_CORES):
        in3.append({"msgs": _permute_msgs(r2[c]["msgs"], plans[c][2], 16),
                    "S": plans[c][1], "invd": plans[c][3], "root": roots[c],
                    "Wc2": Wc2, "b2row": b2row, "ones1": ones1,
                    "id128": id128})
    r3 = _run("L3", lambda: build_segsum(16, True), in3)
    tglob2 = np.zeros((NPAD, 14), dtype=BF16)
    roots2 = []
    for c in range(N_CORES):
        tl = r3[c]["tab2"].reshape(P, NT, 14).transpose(1, 0, 2).reshape(NPC_PAD, 14)
        tglob2[c * NPC:(c + 1) * NPC] = tl[:NPC]
        roots2.append(np.ascontiguousarray(r3[c]["root2v"]))

    # --- L4: weighted gather layer 2
    tabS2 = _stack_table(tglob2, 7)
    in4 = [{"tabS": tabS2, "Bt": plans[c][0]} for c in range(N_CORES)]
    r4 = _run("L4", lambda: build_gather(7), in4)

    # --- L5: segsum + mean + root2 + log_softmax
    in5 = []
    for c in range(N_CORES):
        in5.append({"msgs": _permute_msgs(r4[c]["msgs"], plans[c][2], 7),
                    "S": plans[c][1], "invd": plans[c][3],
                    "root": roots2[c]})
    r5 = _run("L5", lambda: build_segsum(7, False), in5)

    out = np.zeros((N_NODES, F_OUT), dtype=F32)
    for c in range(N_CORES):
        ol = r5[c]["out"].reshape(P, NT, 7).transpose(1, 0, 2).reshape(NPC_PAD, 7)
        out[c * NPC:(c + 1) * NPC] = ol[:NPC]
    return out


# revision 17
# speedup vs baseline: 7.0786x; 1.0389x over previous
"""SplineConv 2-layer GNN (nn_Net_23587960389976) on 8 trn2 NeuronCores.

Structure: 5 SPMD bass launches. All value arithmetic runs on device; the
host only shards, permutes by precomputed indices, and concatenates.

  L1: H = x_shard @ [W1_0|W1_1|root1+b1row]  -> table shard (bf16) + root (f32)
  L2: per-edge basis-weighted gather via fp8 weighted-indicator matmuls:
      64-src-node chunks, lhsT column s holds (1-u_e) at row src%64 and
      u_e at row 64+src%64, rhs = [h0;h1] stacked table chunk -> msg directly.
  L3: windowed segment-sum (32-node dst windows, pure fp8 one-hot scatter
      matmuls packed 4 windows/psum partition group) + mean + root + ELU
      + GEMM2 (PE transposes + matmul, bias via K=1 ones matmul) -> table2/root2
  L4: weighted gather layer 2 (same B matrices, 7-col table)
  L5: segment-sum + mean + root2 + log_softmax

Cost-model-aware choices: matmuls are charged only out-free-size cycles, so
all gather/scatter work rides the PE; DMA is charged per-partition bytes on
the issuing engine queue, so bulk traffic is fp8 and round-robined across the
three DMA-capable queues (SP/sync, Pool/gpsimd, Act/scalar); per-instruction
vector/scalar engine overhead (~60-185ns) is amortized by batching all
DVE/Act ops over >=512-element tiles.

Per-core edge schedule is SPMD-uniform with fixed capacities:
  gather: 3 tiles of 128 slots per 64-src-chunk (384 >= max 320 on seed-0)
  segsum: 10 tiles of 128 slots per 32-dst-window (1280 >= max 1115)
"""
import sys

sys.path.insert(0, "/opt/trn_rl_repo")

import numpy as np
import ml_dtypes

import concourse.bass as bass
import concourse.mybir as mybir

BF16 = ml_dtypes.bfloat16
F8 = ml_dtypes.float8_e3m4
F32 = np.float32

N_NODES = 50000
N_EDGES = 1600000
F_IN, F_HID, F_OUT = 1433, 16, 7
N_CORES = 8
P = 128
NPC = N_NODES // N_CORES           # 6250
NT = 49                            # node tiles per core
NPC_PAD = NT * P                   # 6272
KPAD = 1536                        # 1433 + bias row, padded to 12*128
KT = 12                            # k-chunks in L1
NPAD = 50816                       # 397*128 = 794*64 (global padded nodes)
GCH = 64                           # gather chunk (src nodes)
N_CH = NPAD // GCH                 # 794
N_CH_PAD = 800
TG = 3                             # gather tiles per chunk
GW = (128, 128, 96)                # gather tile widths (slots)
GOFF = (0, 128, 256)               # slot offsets within chunk
CAPG = sum(GW)                     # 352 (seed-0 max is 320)
N_SLOT = N_CH_PAD * CAPG           # 281600 flat B columns
NG = N_CH_PAD * TG                 # 2400 gather (msg) tiles
CB = 32                            # chunks per gather DMA batch (25 batches)
MSC = 4.0                          # layer-1 table scale (fp8 msg headroom)
MSC2 = 64.0                        # layer-2 table scale
WIN = 32                           # scatter window (dst nodes)
N_WIN = NPC_PAD // WIN             # 196
T1 = 10                            # scatter tiles per window
CAPS = T1 * P                      # 1280
N_S = N_WIN * T1                   # 1960 scatter tiles
SB = 160                           # scatter tiles per DMA batch

# ------------------------------------------------------------------ patches
import concourse.tile as tile_mod
from concourse.tile import TileContext
from concourse.vector_clock import ScopedClock


def _patched_drain_and_barrier(self, tick_clock, wait_clock):
    nc = self.nc
    probe = nc.sync.nop(nofuse=True, hint="drain_wait_probe")
    wait_clock.add_sem_waits(probe.ins, ScopedClock({None: tick_clock.global_clock}))
    si = probe.ins.sync_info
    waits = list(si.on_wait) if si is not None else []
    if len(waits) > 1:
        probe.ins.sync_info = mybir.SyncInfo(on_update=list(si.on_update),
                                             on_wait=waits[:1])
        for w in waits[1:]:
            extra = nc.sync.nop(nofuse=True, hint="drain_wait_spill")
            extra.ins.sync_info = mybir.SyncInfo(on_update=[], on_wait=[w])
    nc.sync.drain()
    nc.all_engine_barrier()
    assert self.sems is not None
    popped = nc._tile_sem_poison_stack.pop()
    assert popped is self._sem_poison
    nc.clear_and_free_semaphores(list(self.sems.allocated().values()))
    nc.all_engine_barrier()


tile_mod.TileContext._drain_and_barrier = _patched_drain_and_barrier

_orig_lower = tile_mod.TileContext._lower_ordered_insts


def _split_multi_waits(ordered):
    for insts in ordered.values():
        out = []
        for inst in insts:
            si = getattr(inst, "sync_info", None)
            waits = list(si.on_wait) if si is not None and si.on_wait else []
            if len(waits) > 1:
                for k, w in enumerate(waits[:-1]):
                    out.append(mybir.InstNoOp(
                        name=f"{inst.name}-wsplit{k}", engine=inst.engine,
                        bass_nofuse=True,
                        sync_info=mybir.SyncInfo(on_wait=[w], on_update=[])))
                inst.sync_info = mybir.SyncInfo(on_wait=[waits[-1]],
                                                on_update=list(si.on_update))
            out.append(inst)
        insts[:] = out


def _patched_lower(self, ordered):
    _split_multi_waits(ordered)
    return _orig_lower(self, ordered)


tile_mod.TileContext._lower_ordered_insts = _patched_lower

# ------------------------------------------------------------------ launches


def build_L1():
    nc = bass.Bass()
    xTr = nc.dram_tensor("xTr", [P, NT * KT * P], mybir.dt.bfloat16,
                         kind="ExternalInput")
    Wcp = nc.dram_tensor("Wcp", [P, KT * 48], mybir.dt.bfloat16,
                         kind="ExternalInput")
    table = nc.dram_tensor("table", [P, NT * 32], mybir.dt.bfloat16,
                           kind="ExternalOutput")
    root = nc.dram_tensor("root", [P, NT * 16], mybir.dt.float32,
                          kind="ExternalOutput")
    with TileContext(nc) as tc:
        with tc.tile_pool(name="w", bufs=1) as wpool, \
             tc.tile_pool(name="x", bufs=12) as xpool, \
             tc.tile_pool(name="o", bufs=2) as opool, \
             tc.tile_pool(name="ps", bufs=6, space="PSUM") as pspool:
            engs = [nc.sync, nc.gpsimd, nc.scalar]
            wt = wpool.tile([P, KT, 48], mybir.dt.bfloat16)
            nc.sync.dma_start(out=wt[:],
                              in_=Wcp[:].rearrange("p (a f) -> p a f", f=48))
            XB = 2                  # node tiles per xt DMA
            groups = [(g0, min(5, NT - g0)) for g0 in range(0, NT, 5)]
            qi = 0
            for g0, gn in groups:
                ps = pspool.tile([P, 5, 48], mybir.dt.float32, tag="ps")
                tb = opool.tile([P, 5, 32], mybir.dt.bfloat16, tag="tb")
                rt = opool.tile([P, 5, 16], mybir.dt.float32, tag="rt")
                for j0 in range(0, gn, XB):
                    nx = min(XB, gn - j0)
                    t = g0 + j0
                    xt = xpool.tile([P, XB, KT, P], mybir.dt.bfloat16, tag="xt")
                    engs[qi % 3].dma_start(
                        out=xt[:, 0:nx, :, :],
                        in_=xTr[:, t * KT * P:(t + nx) * KT * P].rearrange(
                            "p (b a n) -> p b a n", a=KT, n=P))
                    qi += 1
                    for i in range(nx):
                        for k in range(KT):
                            nc.tensor.matmul(out=ps[:, j0 + i, :],
                                             lhsT=xt[:, i, k, :],
                                             rhs=wt[:, k, :],
                                             start=(k == 0), stop=(k == KT - 1))
                nc.scalar.copy(out=tb[:, 0:gn, :], in_=ps[:, 0:gn, 0:32])
                nc.vector.tensor_copy(out=rt[:, 0:gn, :], in_=ps[:, 0:gn, 32:48])
                engs[qi % 3].dma_start(
                    out=table[:, g0 * 32:(g0 + gn) * 32],
                    in_=tb[:, 0:gn, :].rearrange("p a f -> p (a f)"))
                qi += 1
                engs[qi % 3].dma_start(
                    out=root[:, g0 * 16:(g0 + gn) * 16],
                    in_=rt[:, 0:gn, :].rearrange("p a f -> p (a f)"))
                qi += 1
    return nc


def build_gather(fdim):
    """L2 (fdim=16) / L4 (fdim=7): weighted-indicator gather.

    msg[slot] = (1-u)*h0[src] + u*h1[src] via one matmul per slot tile:
    lhsT = flat B slice [128, w] fp8 (rows 0:64 carry 1-u at src%64, rows
    64:128 carry u), rhs = stacked table chunk [128, fdim] bf16. Chunk slot
    capacity 352 split (128, 128, 96); per batch of 32 chunks the tiles are
    grouped by width class so psum rows 96:128 of the narrow group are
    simply never copied (dead msg rows are never referenced downstream).
    msg-tile order: batch*96 + width_class*32 + chunk_within_batch.
    """
    nc = bass.Bass()
    tabS = nc.dram_tensor("tabS", [P, N_CH_PAD * fdim], mybir.dt.bfloat16,
                          kind="ExternalInput")
    Bt = nc.dram_tensor("Bt", [P, N_SLOT], mybir.dt.float8e3,
                        kind="ExternalInput")
    msgs = nc.dram_tensor("msgs", [P, NG * fdim], mybir.dt.float8e3,
                          kind="ExternalOutput")
    # chunk batches: two small warmup batches so the PE starts early, then
    # 32-chunk batches. tab is split into per-range tiles so early batches
    # depend only on the first range.
    cbatches = [(0, 8), (8, 24)]
    c = 32
    while c < N_CH_PAD:
        cbatches.append((c, 32))
        c += 32
    tcuts = [(0, 32), (32, 416), (416, N_CH_PAD)]
    with TileContext(nc) as tc:
        with tc.tile_pool(name="tab", bufs=1) as tpool, \
             tc.tile_pool(name="b", bufs=6) as bpool, \
             tc.tile_pool(name="m", bufs=6) as mpool, \
             tc.tile_pool(name="ps", bufs=8, space="PSUM") as pspool:
            engs = [nc.sync, nc.gpsimd, nc.scalar]
            tabt = []
            for k, (lo, hi) in enumerate(tcuts):
                tt = tpool.tile([P, hi - lo, fdim], mybir.dt.bfloat16,
                                name=f"tab{k}")
                engs[(k + 1) % 3].dma_start(
                    out=tt[:],
                    in_=tabS[:, lo * fdim:hi * fdim].rearrange(
                        "p (a f) -> p a f", f=fdim))
                tabt.append((lo, hi, tt))

            def tab_ap(c):
                for lo, hi, tt in tabt:
                    if lo <= c < hi:
                        return tt[:, c - lo, :]

            qi = 0
            for c0, ncb in cbatches:
                bt = bpool.tile([P, CB * CAPG], mybir.dt.float8e3, tag="bt")
                engs[qi % 3].dma_start(
                    out=bt[:, 0:ncb * CAPG],
                    in_=Bt[:, c0 * CAPG:(c0 + ncb) * CAPG])
                qi += 1
                mt = mpool.tile([P, 3 * CB, fdim], mybir.dt.float8e3, tag="mt")
                for k in range(3):
                    w = GW[k]
                    ps = pspool.tile([P, CB, fdim], mybir.dt.float32, tag="ps")
                    for j in range(ncb):
                        sb = j * CAPG + GOFF[k]
                        nc.tensor.matmul(out=ps[0:w, j, :],
                                         lhsT=bt[:, sb:sb + w],
                                         rhs=tab_ap(c0 + j),
                                         start=True, stop=True)
                    nc.vector.tensor_copy(
                        out=mt[0:w, k * ncb:(k + 1) * ncb, :],
                        in_=ps[0:w, 0:ncb, :])
                engs[qi % 3].dma_start(
                    out=msgs[:, c0 * 3 * fdim:(c0 + ncb) * 3 * fdim],
                    in_=mt[:, 0:3 * ncb, :].rearrange("p a c -> p (a c)"))
                qi += 1
    return nc


def build_segsum(fdim, layer1):
    """L3 (fdim=16, layer1) / L5 (fdim=7): windowed segment-sum + tail.

    Scatter matmuls: lhsT = pure one-hot [128 slots, 32] fp8, rhs = msg tile
    [128, fdim] bf16, accumulated T1 per window; window w lands at psum
    partitions 32*(w%4) and free slot w//4 so node n sits at [n%128, n//128].
    """
    nc = bass.Bass()
    msgs = nc.dram_tensor("msgs", [P, N_S * fdim], mybir.dt.float8e3,
                          kind="ExternalInput")
    Sv = nc.dram_tensor("S", [P, N_S * WIN], mybir.dt.float8e3,
                        kind="ExternalInput")
    invd = nc.dram_tensor("invd", [P, NT], mybir.dt.float32,
                          kind="ExternalInput")
    root = nc.dram_tensor("root", [P, NT * fdim], mybir.dt.float32,
                          kind="ExternalInput")
    if layer1:
        Wc2 = nc.dram_tensor("Wc2", [16, 21], mybir.dt.bfloat16,
                             kind="ExternalInput")
        b2row = nc.dram_tensor("b2row", [1, 21], mybir.dt.bfloat16,
                               kind="ExternalInput")
        ones1 = nc.dram_tensor("ones1", [1, P], mybir.dt.bfloat16,
                               kind="ExternalInput")
        id128 = nc.dram_tensor("id128", [P, P], mybir.dt.bfloat16,
                               kind="ExternalInput")
        tab2 = nc.dram_tensor("tab2", [P, NT * 14], mybir.dt.bfloat16,
                              kind="ExternalOutput")
        root2v = nc.dram_tensor("root2v", [P, NT * 7], mybir.dt.float32,
                                kind="ExternalOutput")
    else:
        out = nc.dram_tensor("out", [P, NT * 7], mybir.dt.float32,
                             kind="ExternalOutput")
    # psum agg: one bank per half (slots 0..31 = windows 0..127, slots
    # 32..48 = windows 128..195) so each half's tail overlaps the other
    # half's scatter stream.
    acols = 16 if fdim == 16 else 8
    halves = [(0, 32), (32, NT)]
    with TileContext(nc) as tc:
        with tc.tile_pool(name="sc", bufs=1) as scpool, \
             tc.tile_pool(name="m", bufs=3) as mpool, \
             tc.tile_pool(name="s", bufs=3) as spool, \
             tc.tile_pool(name="h", bufs=1) as hpool, \
             tc.tile_pool(name="tmp", bufs=1) as tmppool, \
             tc.tile_pool(name="psA", bufs=1, space="PSUM") as psApool, \
             tc.tile_pool(name="psB", bufs=1, space="PSUM") as psBpool, \
             tc.tile_pool(name="psT", bufs=2, space="PSUM") as psTpool, \
             tc.tile_pool(name="ps2", bufs=2, space="PSUM") as ps2pool:
            engs = [nc.sync, nc.gpsimd, nc.scalar]
            invt = scpool.tile([P, NT], mybir.dt.float32, tag="invt")
            nc.sync.dma_start(out=invt[:], in_=invd[:])
            roott = scpool.tile([P, NT, fdim], mybir.dt.float32, tag="roott")
            nc.scalar.dma_start(out=roott[:],
                                in_=root[:].rearrange("p (a f) -> p a f", f=fdim))
            if layer1:
                w2t = scpool.tile([16, 21], mybir.dt.bfloat16, tag="w2t")
                nc.gpsimd.dma_start(out=w2t[:], in_=Wc2[:])
                b2t = scpool.tile([1, 21], mybir.dt.bfloat16, tag="b2t")
                nc.gpsimd.dma_start(out=b2t[:], in_=b2row[:])
                onet = scpool.tile([1, P], mybir.dt.bfloat16, tag="onet")
                nc.gpsimd.dma_start(out=onet[:], in_=ones1[:])
                idt = scpool.tile([P, P], mybir.dt.bfloat16, tag="idt")
                nc.gpsimd.dma_start(out=idt[:], in_=id128[:])
            aggs = [psApool.tile([P, 32, acols], mybir.dt.float32,
                                 name="aggA"),
                    psBpool.tile([P, 32, acols], mybir.dt.float32,
                                 name="aggB")]

            def tail(h):
                lo, hi = halves[h]
                ns = hi - lo
                agg = aggs[h]
                hpre = hpool.tile([P, ns, fdim], mybir.dt.float32,
                                  tag=f"hpre{h}")
                nc.vector.tensor_tensor(
                    out=hpre[:], in0=agg[:, 0:ns, 0:fdim],
                    in1=invt[:, lo:hi].to_broadcast([P, ns, fdim]),
                    op=mybir.AluOpType.mult)
                nc.vector.tensor_add(out=hpre[:], in0=hpre[:],
                                     in1=roott[:, lo:hi, :])
                if layer1:
                    # ELU
                    mneg = tmppool.tile([P, ns, 16], mybir.dt.float32,
                                        tag=f"mn{h}")
                    nc.vector.tensor_scalar(out=mneg[:], in0=hpre[:],
                                            scalar1=0.0, scalar2=None,
                                            op0=mybir.AluOpType.min)
                    emt = tmppool.tile([P, ns, 16], mybir.dt.float32,
                                       tag=f"em{h}")
                    nc.scalar.activation(emt[:], mneg[:],
                                         mybir.ActivationFunctionType.Exp)
                    rlu = tmppool.tile([P, ns, 16], mybir.dt.float32,
                                       tag=f"rl{h}")
                    nc.vector.tensor_scalar(out=rlu[:], in0=hpre[:],
                                            scalar1=0.0, scalar2=None,
                                            op0=mybir.AluOpType.max)
                    h1 = hpool.tile([P, ns, 16], mybir.dt.bfloat16,
                                    tag=f"h1{h}")
                    nc.vector.scalar_tensor_tensor(
                        out=h1[:], in0=emt[:], scalar=-1.0, in1=rlu[:],
                        op0=mybir.AluOpType.add, op1=mybir.AluOpType.add)
                    # transposes: h1 [128, t, 16] -> h1T [16, t, 128]
                    h1T = hpool.tile([16, ns, P], mybir.dt.bfloat16,
                                     tag=f"h1T{h}")
                    for t8 in range(0, ns, 8):
                        n8 = min(8, ns - t8)
                        psT = psTpool.tile([16, 8, P], mybir.dt.bfloat16,
                                           tag="psT")
                        for k in range(n8):
                            nc.tensor.transpose(out=psT[:, k, :],
                                                in_=h1[:, t8 + k, :],
                                                identity=idt[:])
                        nc.vector.tensor_copy(out=h1T[:, t8:t8 + n8, :],
                                              in_=psT[:, 0:n8, :])
                    # GEMM2: out = h1 @ [W2_0|W2_1|root2] + [0|0|b2]
                    t2 = hpool.tile([P, ns, 14], mybir.dt.bfloat16,
                                    tag=f"t2{h}")
                    r2v = hpool.tile([P, ns, 7], mybir.dt.float32,
                                     tag=f"r2v{h}")
                    for t24 in range(0, ns, 24):
                        n24 = min(24, ns - t24)
                        ps2 = ps2pool.tile([P, 24, 21], mybir.dt.float32,
                                           tag="ps2")
                        for k in range(n24):
                            nc.tensor.matmul(out=ps2[:, k, :], lhsT=onet[:],
                                             rhs=b2t[:], start=True,
                                             stop=False)
                            nc.tensor.matmul(out=ps2[:, k, :],
                                             lhsT=h1T[:, t24 + k, :],
                                             rhs=w2t[:], start=False,
                                             stop=True)
                        nc.scalar.copy(out=t2[:, t24:t24 + n24, :],
                                       in_=ps2[:, 0:n24, 0:14])
                        nc.vector.tensor_copy(out=r2v[:, t24:t24 + n24, :],
                                              in_=ps2[:, 0:n24, 14:21])
                    nc.sync.dma_start(
                        out=tab2[:, lo * 14:hi * 14],
                        in_=t2[:].rearrange("p a f -> p (a f)"))
                    nc.gpsimd.dma_start(
                        out=root2v[:, lo * 7:hi * 7],
                        in_=r2v[:].rearrange("p a f -> p (a f)"))
                else:
                    # log_softmax over the 7 logits
                    mx = tmppool.tile([P, ns], mybir.dt.float32, tag=f"mx{h}")
                    nc.vector.tensor_reduce(out=mx[:], in_=hpre[:],
                                            axis=mybir.AxisListType.X,
                                            op=mybir.AluOpType.max)
                    z = tmppool.tile([P, ns, 7], mybir.dt.float32,
                                     tag=f"z{h}")
                    nc.vector.tensor_sub(out=z[:], in0=hpre[:],
                                         in1=mx[:].to_broadcast([P, ns, 7]))
                    ez = tmppool.tile([P, ns, 7], mybir.dt.float32,
                                      tag=f"ez{h}")
                    nc.scalar.activation(ez[:], z[:],
                                         mybir.ActivationFunctionType.Exp)
                    se = tmppool.tile([P, ns], mybir.dt.float32, tag=f"se{h}")
                    nc.vector.tensor_reduce(out=se[:], in_=ez[:],
                                            axis=mybir.AxisListType.X,
                                            op=mybir.AluOpType.add)
                    ls = tmppool.tile([P, ns], mybir.dt.float32, tag=f"ls{h}")
                    nc.scalar.activation(ls[:], se[:],
                                         mybir.ActivationFunctionType.Ln)
                    ot = tmppool.tile([P, ns, 7], mybir.dt.float32,
                                      tag=f"ot{h}")
                    nc.vector.tensor_sub(out=ot[:], in0=z[:],
                                         in1=ls[:].to_broadcast([P, ns, 7]))
                    nc.sync.dma_start(
                        out=out[:, lo * 7:hi * 7],
                        in_=ot[:].rearrange("p a f -> p (a f)"))

            # ---- streamed segment-sum, half-A tail issued mid-stream
            qi = 0
            half_a_done = 32 * 4 * T1    # first tile index owned by half B
            for s0 in range(0, N_S, SB):
                nb = min(SB, N_S - s0)
                mt = mpool.tile([P, SB, fdim], mybir.dt.float8e3, tag="mt")
                engs[qi % 3].dma_start(
                    out=mt[:, 0:nb, :],
                    in_=msgs[:, s0 * fdim:(s0 + nb) * fdim].rearrange(
                        "p (a c) -> p a c", c=fdim))
                qi += 1
                st = spool.tile([P, SB, WIN], mybir.dt.float8e3, tag="st")
                engs[qi % 3].dma_start(
                    out=st[:, 0:nb, :],
                    in_=Sv[:, s0 * WIN:(s0 + nb) * WIN].rearrange(
                        "p (a c) -> p a c", c=WIN))
                qi += 1
                for j in range(nb):
                    t = s0 + j
                    w, tw = divmod(t, T1)
                    a, q = divmod(w, 4)
                    agg = aggs[0] if a < 32 else aggs[1]
                    dst = agg[32 * q:32 * q + 32, a % 32, 0:fdim]
                    nc.tensor.matmul(out=dst, lhsT=st[:, j, :],
                                     rhs=mt[:, j, 0:fdim],
                                     start=(tw == 0), stop=(tw == T1 - 1),
                                     tile_position=(0, 32 * q))
                if s0 < half_a_done <= s0 + nb:
                    tail(0)
            tail(1)
    return nc


# ------------------------------------------------------------------ host prep


def _rank_within_group(group_sorted):
    n = group_sorted.shape[0]
    if n == 0:
        return np.zeros(0, dtype=np.int64)
    first = np.searchsorted(group_sorted, group_sorted, side="left")
    return np.arange(n, dtype=np.int64) - first


def plan_core(src, dst_local, u):
    E = src.shape[0]
    # gather side (src-sorted, 64-node chunks, flat 352-slot capacity)
    og = np.argsort(src, kind="stable")
    sg = src[og]
    chunk = sg // GCH
    rank = _rank_within_group(chunk)
    assert rank.max(initial=0) < CAPG, "gather chunk overflow"
    slot = chunk * CAPG + rank         # flat B column
    r = sg - chunk * GCH
    uo = u[og].astype(F32)
    Bt = np.zeros((P, N_SLOT), dtype=F8)
    Bt[r, slot] = (1.0 - uo).astype(F8)
    Bt[r + GCH, slot] = uo.astype(F8)
    # flat msg position: tiles grouped per 32-chunk batch by width class
    kcl = np.minimum(rank // 128, 2)
    row = rank - kcl * 128
    mtile = (chunk // CB) * (3 * CB) + kcl * CB + (chunk % CB)
    flat = mtile * P + row
    slot_of_edge = np.empty(E, dtype=np.int64)
    slot_of_edge[og] = flat
    # segsum side (dst-sorted, 32-node windows)
    os_ = np.argsort(dst_local, kind="stable")
    ds = dst_local[os_]
    win = ds // WIN
    rank_s = _rank_within_group(win)
    assert rank_s.max(initial=0) < CAPS, "segsum window overflow"
    pos = win * CAPS + rank_s          # == tile*128 + row
    st_ = pos // P
    sr = pos % P
    Sm = np.zeros((P, N_S * WIN), dtype=F8)
    Sm[sr, st_ * WIN + (ds - win * WIN)] = F8(1.0)
    perm = np.zeros((P, N_S), dtype=np.int64)
    perm[sr, st_] = slot_of_edge[os_]
    deg = np.bincount(dst_local, minlength=NPC).astype(F32)
    inv_pad = np.zeros(NPC_PAD, dtype=F32)
    inv_pad[:NPC] = 1.0 / np.clip(deg, 1.0, None)
    invd = np.ascontiguousarray(inv_pad.reshape(NT, P).T)
    return Bt, Sm, perm, invd


def _permute_msgs(gmsgs, perm, fdim):
    """gather msgs [P, NG*fdim] -> scatter-slot layout [P, N_S*fdim]."""
    flat = np.ascontiguousarray(
        gmsgs.reshape(P, NG, fdim).transpose(1, 0, 2)).reshape(NG * P, fdim)
    mp = flat[perm]                    # [P, N_S, fdim]
    return np.ascontiguousarray(mp).reshape(P, N_S * fdim)


def _stack_table(tglob, fdim):
    """[NPAD, 2*fdim] -> stacked gather table [P, N_CH_PAD*fdim]."""
    m = np.arange(NPAD)
    ck, ri = m // GCH, m % GCH
    tabS = np.zeros((P, N_CH_PAD, fdim), dtype=BF16)
    tabS[ri, ck] = tglob[:, 0:fdim]
    tabS[ri + GCH, ck] = tglob[:, fdim:2 * fdim]
    return np.ascontiguousarray(tabS).reshape(P, N_CH_PAD * fdim)


# ------------------------------------------------------------------ driver


_NC_CACHE = {}


def _get_nc(name, builder):
    if name not in _NC_CACHE:
        _NC_CACHE[name] = builder()
    return _NC_CACHE[name]


def _run(name, builder, in_maps):
    from concourse.bass_utils import run_bass_kernel_spmd
    import time
    nc = _get_nc(name, builder)
    t0 = time.time()
    res = run_bass_kernel_spmd(nc, in_maps, list(range(N_CORES)))
    _run.times[name] = time.time() - t0
    return res.results


_run.times = {}


def kernel(x, edge_attr, edge_index, W1, root1, b1, W2, root2, b2):
    import os
    dbg = bool(os.environ.get("KERNEL_DEBUG"))
    x = np.asarray(x, dtype=F32)
    u = np.asarray(edge_attr, dtype=F32).reshape(-1)
    ei = np.asarray(edge_index, dtype=np.int64)
    src_all, dst_all = ei[0], ei[1]

    # --- shard edges by dst owner core
    owner = dst_all // NPC
    plans = []
    for c in range(N_CORES):
        m = owner == c
        plans.append(plan_core(src_all[m], dst_all[m] - c * NPC, u[m]))

    # --- L1: GEMM (x @ [W1_0|W1_1|root1], bias row for root part)
    Wc = np.zeros((KPAD, 48), dtype=F32)
    Wc[:F_IN, 0:16] = np.asarray(W1[0], dtype=F32) * MSC
    Wc[:F_IN, 16:32] = np.asarray(W1[1], dtype=F32) * MSC
    Wc[:F_IN, 32:48] = np.asarray(root1, dtype=F32)
    Wc[F_IN, 32:48] = np.asarray(b1, dtype=F32)
    Wcp = np.ascontiguousarray(
        Wc.reshape(KT, P, 48).transpose(1, 0, 2)).reshape(P, KT * 48).astype(BF16)
    in1 = []
    for c in range(N_CORES):
        xf = np.zeros((NPC_PAD, KPAD), dtype=BF16)
        xf[:NPC, :F_IN] = x[c * NPC:(c + 1) * NPC].astype(BF16)
        xf[:NPC, F_IN] = BF16(1.0)
        xTr = np.ascontiguousarray(
            xf.reshape(NT, P, KT, P).transpose(3, 0, 2, 1)).reshape(P, NT * KT * P)
        in1.append({"xTr": xTr, "Wcp": Wcp})
    r1 = _run("L1", build_L1, in1)
    tglob1 = np.zeros((NPAD, 32), dtype=BF16)
    roots = []
    for c in range(N_CORES):
        tl = r1[c]["table"].reshape(P, NT, 32).transpose(1, 0, 2).reshape(NPC_PAD, 32)
        tglob1[c * NPC:(c + 1) * NPC] = tl[:NPC]
        roots.append(np.ascontiguousarray(r1[c]["root"]))
    if dbg:
        xfull = np.zeros((N_NODES, KPAD), dtype=F32)
        xfull[:, :F_IN] = x
        xfull[:, F_IN] = 1.0
        Hexp = xfull @ Wc
        got = tglob1[:N_NODES].astype(F32)
        print("L1 table relerr:",
              np.abs(got - Hexp[:, 0:32]).max() / np.abs(Hexp[:, 0:32]).max())
        r0 = roots[0].reshape(P, NT, 16).transpose(1, 0, 2).reshape(NPC_PAD, 16)
        print("L1 root relerr:",
              np.abs(r0[:NPC] - Hexp[:NPC, 32:48]).max() / np.abs(Hexp[:, 32:48]).max())

    # --- L2: weighted gather layer 1
    tabS1 = _stack_table(tglob1, 16)
    in2 = [{"tabS": tabS1, "Bt": plans[c][0]} for c in range(N_CORES)]
    r2 = _run("L2", lambda: build_gather(16), in2)
    if dbg:
        c = 0
        m = owner == c
        s0, u0 = src_all[m], u[m]
        og = np.argsort(s0, kind="stable")
        sg = s0[og]
        ch = sg // GCH
        rk = _rank_within_group(ch)
        kcl = np.minimum(rk // 128, 2)
        mtile = (ch // CB) * (3 * CB) + kcl * CB + (ch % CB)
        row = rk - kcl * 128
        tabf = tglob1.astype(F32)
        exp_msg = ((1 - u0[og])[:, None] * tabf[sg, 0:16]
                   + u0[og][:, None] * tabf[sg, 16:32])
        gm = r2[c]["msgs"].reshape(P, NG, 16)
        got = gm[row, mtile].astype(F32)
        print("L2 msg relerr:",
              np.abs(got - exp_msg).max() / np.abs(exp_msg).max())

    # --- L3: segsum + mean + root + ELU + GEMM2
    Wc2 = np.zeros((16, 21), dtype=BF16)
    Wc2[:, 0:7] = np.asarray(np.asarray(W2[0], dtype=F32) * MSC2, dtype=BF16)
    Wc2[:, 7:14] = np.asarray(np.asarray(W2[1], dtype=F32) * MSC2, dtype=BF16)
    Wc2[:, 14:21] = np.asarray(root2, dtype=BF16)
    b2row = np.zeros((1, 21), dtype=BF16)
    b2row[0, 14:21] = np.asarray(b2, dtype=BF16)
    ones1 = np.ones((1, P), dtype=BF16)
    id128 = np.eye(P, dtype=BF16)
    in3 = []
    for c in range(N_CORES):
        in3.append({"msgs": _permute_msgs(r2[c]["msgs"], plans[c][2], 16),
                    "S": plans[c][1], "invd": plans[c][3] / MSC,
                    "root": roots[c],
                    "Wc2": Wc2, "b2row": b2row, "ones1": ones1,
                    "id128": id128})
    r3 = _run("L3", lambda: build_segsum(16, True), in3)
    tglob2 = np.zeros((NPAD, 14), dtype=BF16)
    roots2 = []
    for c in range(N_CORES):
        tl = r3[c]["tab2"].reshape(P, NT, 14).transpose(1, 0, 2).reshape(NPC_PAD, 14)
        tglob2[c * NPC:(c + 1) * NPC] = tl[:NPC]
        roots2.append(np.ascontiguousarray(r3[c]["root2v"]))

    # --- L4: weighted gather layer 2
    tabS2 = _stack_table(tglob2, 7)
    in4 = [{"tabS": tabS2, "Bt": plans[c][0]} for c in range(N_CORES)]
    r4 = _run("L4", lambda: build_gather(7), in4)

    # --- L5: segsum + mean + root2 + log_softmax
    in5 = []
    for c in range(N_CORES):
        in5.append({"msgs": _permute_msgs(r4[c]["msgs"], plans[c][2], 7),
                    "S": plans[c][1], "invd": plans[c][3] / MSC2,
                    "root": roots2[c]})
    r5 = _run("L5", lambda: build_segsum(7, False), in5)

    out = np.zeros((N_NODES, F_OUT), dtype=F32)
    for c in range(N_CORES):
        ol = r5[c]["out"].reshape(P, NT, 7).transpose(1, 0, 2).reshape(NPC_PAD, 7)
        out[c * NPC:(c + 1) * NPC] = ol[:NPC]
    return out


# revision 29
# speedup vs baseline: 7.7051x; 1.0885x over previous
"""SplineConv 2-layer GNN (nn_Net_23587960389976) on 8 trn2 NeuronCores.

Structure: 5 SPMD bass launches. All value arithmetic runs on device; the
host only shards, permutes by precomputed indices, and concatenates.

  L1: H = x_shard @ [W1_0|W1_1|root1+b1row]  -> table shard (bf16) + root (f32)
  L2: per-edge basis-weighted gather via fp8 weighted-indicator matmuls:
      64-src-node chunks, lhsT column s holds (1-u_e) at row src%64 and
      u_e at row 64+src%64, rhs = [h0;h1] stacked table chunk -> msg directly.
  L3: windowed segment-sum (32-node dst windows, pure fp8 one-hot scatter
      matmuls packed 4 windows/psum partition group) + mean + root + ELU
      + GEMM2 (PE transposes + matmul, bias via K=1 ones matmul) -> table2/root2
  L4: weighted gather layer 2 (same B matrices, 7-col table)
  L5: segment-sum + mean + root2 + log_softmax

Cost-model-aware choices: matmuls are charged only out-free-size cycles, so
all gather/scatter work rides the PE; DMA is charged per-partition bytes on
the issuing engine queue, so bulk traffic is fp8 and round-robined across the
three DMA-capable queues (SP/sync, Pool/gpsimd, Act/scalar); per-instruction
vector/scalar engine overhead (~60-185ns) is amortized by batching all
DVE/Act ops over >=512-element tiles.

Per-core edge schedule is SPMD-uniform with fixed capacities:
  gather: 3 tiles of 128 slots per 64-src-chunk (384 >= max 320 on seed-0)
  segsum: 10 tiles of 128 slots per 32-dst-window (1280 >= max 1115)
"""
import sys

sys.path.insert(0, "/opt/trn_rl_repo")

import numpy as np
import ml_dtypes

import concourse.bass as bass
import concourse.mybir as mybir

BF16 = ml_dtypes.bfloat16
F8 = ml_dtypes.float8_e3m4
F32 = np.float32

N_NODES = 50000
N_EDGES = 1600000
F_IN, F_HID, F_OUT = 1433, 16, 7
N_CORES = 8
P = 128
NPC = N_NODES // N_CORES           # 6250
NT = 49                            # node tiles per core
NPC_PAD = NT * P                   # 6272
KPAD = 1536                        # 1433 + bias row, padded to 12*128
KT = 12                            # k-chunks in L1
NPAD = 50816                       # 397*128 = 794*64 (global padded nodes)
GCH = 64                           # gather chunk (src nodes)
N_CH = NPAD // GCH                 # 794
N_CH_PAD = 800
TG = 3                             # gather tiles per chunk
GW = (128, 128, 96)                # gather tile widths (slots)
GOFF = (0, 128, 256)               # slot offsets within chunk
CAPG = sum(GW)                     # 352 (seed-0 max is 320)
N_SLOT = N_CH_PAD * CAPG           # 281600 flat B columns
NG = N_CH_PAD * TG                 # 2400 gather (msg) tiles
CB = 32                            # chunks per gather DMA batch
CBATCHES = [(0, 8), (8, 24)]       # warmup batches, CB-chunk body, tapered tail
_c = 32
while _c < N_CH_PAD - 32:
    CBATCHES.append((_c, CB))
    _c += CB
CBATCHES += [(_c, 16), (_c + 16, 8), (_c + 24, 8)]
_BST = np.array([b[0] for b in CBATCHES])
_BSZ = np.array([b[1] for b in CBATCHES])
MSC = 4.0                          # layer-1 table scale (fp8 msg headroom)
MSC2 = 64.0                        # layer-2 table scale
WIN = 32                           # scatter window (dst nodes)
N_WIN = NPC_PAD // WIN             # 196
T1 = 10                            # scatter tiles per window
CAPS = T1 * P                      # 1280
N_S = N_WIN * T1                   # 1960 scatter tiles
SB = 160                           # scatter tiles per DMA batch

# ------------------------------------------------------------------ patches
import concourse.tile as tile_mod
from concourse.tile import TileContext
from concourse.vector_clock import ScopedClock


def _patched_drain_and_barrier(self, tick_clock, wait_clock):
    nc = self.nc
    probe = nc.sync.nop(nofuse=True, hint="drain_wait_probe")
    wait_clock.add_sem_waits(probe.ins, ScopedClock({None: tick_clock.global_clock}))
    si = probe.ins.sync_info
    waits = list(si.on_wait) if si is not None else []
    if len(waits) > 1:
        probe.ins.sync_info = mybir.SyncInfo(on_update=list(si.on_update),
                                             on_wait=waits[:1])
        for w in waits[1:]:
            extra = nc.sync.nop(nofuse=True, hint="drain_wait_spill")
            extra.ins.sync_info = mybir.SyncInfo(on_update=[], on_wait=[w])
    nc.sync.drain()
    nc.all_engine_barrier()
    assert self.sems is not None
    popped = nc._tile_sem_poison_stack.pop()
    assert popped is self._sem_poison
    nc.clear_and_free_semaphores(list(self.sems.allocated().values()))
    nc.all_engine_barrier()


tile_mod.TileContext._drain_and_barrier = _patched_drain_and_barrier

_orig_lower = tile_mod.TileContext._lower_ordered_insts


def _split_multi_waits(ordered):
    for insts in ordered.values():
        out = []
        for inst in insts:
            si = getattr(inst, "sync_info", None)
            waits = list(si.on_wait) if si is not None and si.on_wait else []
            if len(waits) > 1:
                for k, w in enumerate(waits[:-1]):
                    out.append(mybir.InstNoOp(
                        name=f"{inst.name}-wsplit{k}", engine=inst.engine,
                        bass_nofuse=True,
                        sync_info=mybir.SyncInfo(on_wait=[w], on_update=[])))
                inst.sync_info = mybir.SyncInfo(on_wait=[waits[-1]],
                                                on_update=list(si.on_update))
            out.append(inst)
        insts[:] = out


def _patched_lower(self, ordered):
    _split_multi_waits(ordered)
    return _orig_lower(self, ordered)


tile_mod.TileContext._lower_ordered_insts = _patched_lower

# ------------------------------------------------------------------ launches


def build_L1():
    nc = bass.Bass()
    xTr = nc.dram_tensor("xTr", [P, NT * KT * P], mybir.dt.float8e3,
                         kind="ExternalInput")
    Wcp = nc.dram_tensor("Wcp", [P, KT * 48], mybir.dt.bfloat16,
                         kind="ExternalInput")
    table = nc.dram_tensor("table", [P, NT * 32], mybir.dt.bfloat16,
                           kind="ExternalOutput")
    root = nc.dram_tensor("root", [P, NT * 16], mybir.dt.float32,
                          kind="ExternalOutput")
    with TileContext(nc) as tc:
        with tc.tile_pool(name="w", bufs=1) as wpool, \
             tc.tile_pool(name="x", bufs=12) as xpool, \
             tc.tile_pool(name="o", bufs=2) as opool, \
             tc.tile_pool(name="ps", bufs=6, space="PSUM") as pspool:
            engs = [nc.sync, nc.gpsimd, nc.scalar]
            wt = wpool.tile([P, KT, 48], mybir.dt.bfloat16)
            nc.sync.dma_start(out=wt[:],
                              in_=Wcp[:].rearrange("p (a f) -> p a f", f=48))
            XB = 2                  # node tiles per xt DMA
            groups = [(g0, min(5, NT - g0)) for g0 in range(0, NT, 5)]
            qi = 0
            for g0, gn in groups:
                ps = pspool.tile([P, 5, 48], mybir.dt.float32, tag="ps")
                tb = opool.tile([P, 5, 32], mybir.dt.bfloat16, tag="tb")
                rt = opool.tile([P, 5, 16], mybir.dt.float32, tag="rt")
                for j0 in range(0, gn, XB):
                    nx = min(XB, gn - j0)
                    t = g0 + j0
                    xt = xpool.tile([P, XB, KT, P], mybir.dt.float8e3, tag="xt")
                    engs[qi % 3].dma_start(
                        out=xt[:, 0:nx, :, :],
                        in_=xTr[:, t * KT * P:(t + nx) * KT * P].rearrange(
                            "p (b a n) -> p b a n", a=KT, n=P))
                    qi += 1
                    for i in range(nx):
                        for k in range(KT):
                            nc.tensor.matmul(out=ps[:, j0 + i, :],
                                             lhsT=xt[:, i, k, :],
                                             rhs=wt[:, k, :],
                                             start=(k == 0), stop=(k == KT - 1))
                nc.scalar.copy(out=tb[:, 0:gn, :], in_=ps[:, 0:gn, 0:32])
                nc.vector.tensor_copy(out=rt[:, 0:gn, :], in_=ps[:, 0:gn, 32:48])
                engs[qi % 3].dma_start(
                    out=table[:, g0 * 32:(g0 + gn) * 32],
                    in_=tb[:, 0:gn, :].rearrange("p a f -> p (a f)"))
                qi += 1
                engs[qi % 3].dma_start(
                    out=root[:, g0 * 16:(g0 + gn) * 16],
                    in_=rt[:, 0:gn, :].rearrange("p a f -> p (a f)"))
                qi += 1
    return nc


def build_gather(fdim):
    """L2 (fdim=16) / L4 (fdim=7): weighted-indicator gather.

    msg[slot] = (1-u)*h0[src] + u*h1[src] via one matmul per slot tile:
    lhsT = flat B slice [128, w] fp8 (rows 0:64 carry 1-u at src%64, rows
    64:128 carry u), rhs = stacked table chunk [128, fdim] bf16. Chunk slot
    capacity 352 split (128, 128, 96); per batch of 32 chunks the tiles are
    grouped by width class so psum rows 96:128 of the narrow group are
    simply never copied (dead msg rows are never referenced downstream).
    msg-tile order: batch*96 + width_class*32 + chunk_within_batch.
    """
    nc = bass.Bass()
    tabS = nc.dram_tensor("tabS", [P, N_CH_PAD * fdim], mybir.dt.bfloat16,
                          kind="ExternalInput")
    Bt = nc.dram_tensor("Bt", [P, N_SLOT], mybir.dt.float8e3,
                        kind="ExternalInput")
    msgs = nc.dram_tensor("msgs", [P, NG * fdim], mybir.dt.float8e3,
                          kind="ExternalOutput")
    # chunk batches: two small warmup batches so the PE starts early, then
    # 32-chunk batches. tab is split into per-range tiles so early batches
    # depend only on the first range.
    cbatches = list(CBATCHES)
    tcuts = [(0, 32), (32, 416), (416, N_CH_PAD)]
    with TileContext(nc) as tc:
        with tc.tile_pool(name="tab", bufs=1) as tpool, \
             tc.tile_pool(name="b", bufs=6) as bpool, \
             tc.tile_pool(name="m", bufs=6) as mpool, \
             tc.tile_pool(name="ps", bufs=8, space="PSUM") as pspool:
            engs = [nc.sync, nc.gpsimd, nc.scalar]
            tabt = []
            for k, (lo, hi) in enumerate(tcuts):
                tt = tpool.tile([P, hi - lo, fdim], mybir.dt.bfloat16,
                                name=f"tab{k}")
                tabt.append((lo, hi, tt))

            def tab_dma(k, eng):
                lo, hi, tt = tabt[k]
                eng.dma_start(
                    out=tt[:],
                    in_=tabS[:, lo * fdim:hi * fdim].rearrange(
                        "p (a f) -> p a f", f=fdim))

            def tab_ap(c):
                for lo, hi, tt in tabt:
                    if lo <= c < hi:
                        return tt[:, c - lo, :]

            def emit_batch(c0, ncb, b_eng, m_eng):
                bt = bpool.tile([P, CB * CAPG], mybir.dt.float8e3, tag="bt")
                b_eng.dma_start(
                    out=bt[:, 0:ncb * CAPG],
                    in_=Bt[:, c0 * CAPG:(c0 + ncb) * CAPG])
                mt = mpool.tile([P, 3 * CB, fdim], mybir.dt.float8e3, tag="mt")
                for k in range(3):
                    w = GW[k]
                    for j0 in range(0, ncb, 32):
                        nj = min(32, ncb - j0)
                        ps = pspool.tile([P, 32, fdim], mybir.dt.float32,
                                         tag="ps")
                        for j in range(j0, j0 + nj):
                            sb = j * CAPG + GOFF[k]
                            nc.tensor.matmul(out=ps[0:w, j - j0, :],
                                             lhsT=bt[:, sb:sb + w],
                                             rhs=tab_ap(c0 + j),
                                             start=True, stop=True)
                        nc.vector.tensor_copy(
                            out=mt[0:w, k * ncb + j0:k * ncb + j0 + nj, :],
                            in_=ps[0:w, 0:nj, :])
                m_eng.dma_start(
                    out=msgs[:, c0 * 3 * fdim:(c0 + ncb) * 3 * fdim],
                    in_=mt[:, 0:3 * ncb, :].rearrange("p a c -> p (a c)"))

            # warmup: tiny tab range + small B batches dispatch before the
            # bulk tab ranges so the PE/DVE streams start immediately
            tab_dma(0, nc.scalar)
            emit_batch(*cbatches[0], nc.sync, nc.gpsimd)
            emit_batch(*cbatches[1], nc.gpsimd, nc.sync)
            tab_dma(1, nc.scalar)
            tab_dma(2, nc.gpsimd)
            for i, (c0, ncb) in enumerate(cbatches[2:]):
                emit_batch(c0, ncb, engs[i % 3], engs[(i + 1) % 3])
    return nc


def build_segsum(fdim, layer1):
    """L3 (fdim=16, layer1) / L5 (fdim=7): windowed segment-sum + tail.

    Scatter matmuls: lhsT = pure one-hot [128 slots, 32] fp8, rhs = msg tile
    [128, fdim] bf16, accumulated T1 per window; window w lands at psum
    partitions 32*(w%4) and free slot w//4 so node n sits at [n%128, n//128].
    """
    nc = bass.Bass()
    msgs = nc.dram_tensor("msgs", [P, N_S * fdim], mybir.dt.float8e3,
                          kind="ExternalInput")
    Sv = nc.dram_tensor("S", [P, N_S * WIN], mybir.dt.float8e3,
                        kind="ExternalInput")
    invd = nc.dram_tensor("invd", [P, NT], mybir.dt.float32,
                          kind="ExternalInput")
    root = nc.dram_tensor("root", [P, NT * fdim], mybir.dt.float32,
                          kind="ExternalInput")
    if layer1:
        Wc2 = nc.dram_tensor("Wc2", [16, 21], mybir.dt.bfloat16,
                             kind="ExternalInput")
        b2row = nc.dram_tensor("b2row", [1, 21], mybir.dt.bfloat16,
                               kind="ExternalInput")
        ones1 = nc.dram_tensor("ones1", [1, P], mybir.dt.bfloat16,
                               kind="ExternalInput")
        id128 = nc.dram_tensor("id128", [P, P], mybir.dt.bfloat16,
                               kind="ExternalInput")
        tab2 = nc.dram_tensor("tab2", [P, NT * 14], mybir.dt.bfloat16,
                              kind="ExternalOutput")
        root2v = nc.dram_tensor("root2v", [P, NT * 7], mybir.dt.float32,
                                kind="ExternalOutput")
    else:
        out = nc.dram_tensor("out", [P, NT * 7], mybir.dt.float32,
                             kind="ExternalOutput")
    # psum agg: one bank per slot-quarter so each quarter's tail chain
    # overlaps the remaining scatter stream.
    acols = 16 if fdim == 16 else 8
    quarters = [(0, 16), (16, 32), (32, 48), (48, NT)]
    with TileContext(nc) as tc:
        with tc.tile_pool(name="sc", bufs=1) as scpool, \
             tc.tile_pool(name="m", bufs=6) as mpool, \
             tc.tile_pool(name="s", bufs=6) as spool, \
             tc.tile_pool(name="h", bufs=1) as hpool, \
             tc.tile_pool(name="tmp", bufs=1) as tmppool, \
             tc.tile_pool(name="psA", bufs=1, space="PSUM") as psApool, \
             tc.tile_pool(name="psT", bufs=2, space="PSUM") as psTpool, \
             tc.tile_pool(name="ps2", bufs=2, space="PSUM") as ps2pool:
            engs = [nc.sync, nc.gpsimd, nc.scalar]
            invt = scpool.tile([P, NT], mybir.dt.float32, tag="invt")
            nc.sync.dma_start(out=invt[:], in_=invd[:])
            roott = scpool.tile([P, NT, fdim], mybir.dt.float32, tag="roott")
            nc.scalar.dma_start(out=roott[:],
                                in_=root[:].rearrange("p (a f) -> p a f", f=fdim))
            if layer1:
                w2t = scpool.tile([16, 21], mybir.dt.bfloat16, tag="w2t")
                nc.gpsimd.dma_start(out=w2t[:], in_=Wc2[:])
                b2t = scpool.tile([1, 21], mybir.dt.bfloat16, tag="b2t")
                nc.gpsimd.dma_start(out=b2t[:], in_=b2row[:])
                onet = scpool.tile([1, P], mybir.dt.bfloat16, tag="onet")
                nc.gpsimd.dma_start(out=onet[:], in_=ones1[:])
                idt = scpool.tile([P, P], mybir.dt.bfloat16, tag="idt")
                nc.gpsimd.dma_start(out=idt[:], in_=id128[:])
            aggs = [psApool.tile([P, 16, acols], mybir.dt.float32,
                                 name=f"aggQ{k}", tag=f"aggQ{k}")
                    for k in range(4)]

            def tail(h):
                lo, hi = quarters[h]
                ns = hi - lo
                agg = aggs[h]
                hpre = hpool.tile([P, ns, fdim], mybir.dt.float32,
                                  tag=f"hpre{h}")
                nc.vector.tensor_tensor(
                    out=hpre[:], in0=agg[:, 0:ns, 0:fdim],
                    in1=invt[:, lo:hi].to_broadcast([P, ns, fdim]),
                    op=mybir.AluOpType.mult)
                nc.vector.tensor_add(out=hpre[:], in0=hpre[:],
                                     in1=roott[:, lo:hi, :])
                if layer1:
                    # ELU
                    mneg = tmppool.tile([P, ns, 16], mybir.dt.float32,
                                        tag=f"mn{h}")
                    nc.vector.tensor_scalar(out=mneg[:], in0=hpre[:],
                                            scalar1=0.0, scalar2=None,
                                            op0=mybir.AluOpType.min)
                    emt = tmppool.tile([P, ns, 16], mybir.dt.float32,
                                       tag=f"em{h}")
                    nc.scalar.activation(emt[:], mneg[:],
                                         mybir.ActivationFunctionType.Exp)
                    rlu = tmppool.tile([P, ns, 16], mybir.dt.float32,
                                       tag=f"rl{h}")
                    nc.vector.tensor_scalar(out=rlu[:], in0=hpre[:],
                                            scalar1=0.0, scalar2=None,
                                            op0=mybir.AluOpType.max)
                    h1 = hpool.tile([P, ns, 16], mybir.dt.bfloat16,
                                    tag=f"h1{h}")
                    nc.vector.scalar_tensor_tensor(
                        out=h1[:], in0=emt[:], scalar=-1.0, in1=rlu[:],
                        op0=mybir.AluOpType.add, op1=mybir.AluOpType.add)
                    # transposes: h1 [128, t, 16] -> h1T [16, t, 128]
                    h1T = hpool.tile([16, ns, P], mybir.dt.bfloat16,
                                     tag=f"h1T{h}")
                    for t8 in range(0, ns, 8):
                        n8 = min(8, ns - t8)
                        psT = psTpool.tile([16, 8, P], mybir.dt.bfloat16,
                                           tag="psT")
                        for k in range(n8):
                            nc.tensor.transpose(out=psT[:, k, :],
                                                in_=h1[:, t8 + k, :],
                                                identity=idt[:])
                        nc.vector.tensor_copy(out=h1T[:, t8:t8 + n8, :],
                                              in_=psT[:, 0:n8, :])
                    # GEMM2: out = h1 @ [W2_0|W2_1|root2] + [0|0|b2]
                    t2 = hpool.tile([P, ns, 14], mybir.dt.bfloat16,
                                    tag=f"t2{h}")
                    r2v = hpool.tile([P, ns, 7], mybir.dt.float32,
                                     tag=f"r2v{h}")
                    for t24 in range(0, ns, 24):
                        n24 = min(24, ns - t24)
                        ps2 = ps2pool.tile([P, 24, 21], mybir.dt.float32,
                                           tag="ps2")
                        for k in range(n24):
                            nc.tensor.matmul(out=ps2[:, k, :], lhsT=onet[:],
                                             rhs=b2t[:], start=True,
                                             stop=False)
                            nc.tensor.matmul(out=ps2[:, k, :],
                                             lhsT=h1T[:, t24 + k, :],
                                             rhs=w2t[:], start=False,
                                             stop=True)
                        nc.scalar.copy(out=t2[:, t24:t24 + n24, :],
                                       in_=ps2[:, 0:n24, 0:14])
                        nc.vector.tensor_copy(out=r2v[:, t24:t24 + n24, :],
                                              in_=ps2[:, 0:n24, 14:21])
                    nc.sync.dma_start(
                        out=tab2[:, lo * 14:hi * 14],
                        in_=t2[:].rearrange("p a f -> p (a f)"))
                    nc.gpsimd.dma_start(
                        out=root2v[:, lo * 7:hi * 7],
                        in_=r2v[:].rearrange("p a f -> p (a f)"))
                else:
                    # log_softmax over the 7 logits
                    mx = tmppool.tile([P, ns], mybir.dt.float32, tag=f"mx{h}")
                    nc.vector.tensor_reduce(out=mx[:], in_=hpre[:],
                                            axis=mybir.AxisListType.X,
                                            op=mybir.AluOpType.max)
                    z = tmppool.tile([P, ns, 7], mybir.dt.float32,
                                     tag=f"z{h}")
                    nc.vector.tensor_sub(out=z[:], in0=hpre[:],
                                         in1=mx[:].to_broadcast([P, ns, 7]))
                    ez = tmppool.tile([P, ns, 7], mybir.dt.float32,
                                      tag=f"ez{h}")
                    nc.scalar.activation(ez[:], z[:],
                                         mybir.ActivationFunctionType.Exp)
                    se = tmppool.tile([P, ns], mybir.dt.float32, tag=f"se{h}")
                    nc.vector.tensor_reduce(out=se[:], in_=ez[:],
                                            axis=mybir.AxisListType.X,
                                            op=mybir.AluOpType.add)
                    ls = tmppool.tile([P, ns], mybir.dt.float32, tag=f"ls{h}")
                    nc.scalar.activation(ls[:], se[:],
                                         mybir.ActivationFunctionType.Ln)
                    ot = tmppool.tile([P, ns, 7], mybir.dt.float32,
                                      tag=f"ot{h}")
                    nc.vector.tensor_sub(out=ot[:], in0=z[:],
                                         in1=ls[:].to_broadcast([P, ns, 7]))
                    nc.sync.dma_start(
                        out=out[:, lo * 7:hi * 7],
                        in_=ot[:].rearrange("p a f -> p (a f)"))

            # ---- streamed segment-sum; per-quarter tails issued mid-stream
            qdone = [16 * 4 * T1, 32 * 4 * T1, 48 * 4 * T1]
            batches = [(0, 40)]
            s = 40
            while s < N_S:
                batches.append((s, min(SB, N_S - s)))
                s += SB
            qi = 0
            for s0, nb in batches:
                mt = mpool.tile([P, SB, fdim], mybir.dt.float8e3, tag="mt")
                engs[qi % 3].dma_start(
                    out=mt[:, 0:nb, :],
                    in_=msgs[:, s0 * fdim:(s0 + nb) * fdim].rearrange(
                        "p (a c) -> p a c", c=fdim))
                qi += 1
                st = spool.tile([P, SB, WIN], mybir.dt.float8e3, tag="st")
                engs[qi % 3].dma_start(
                    out=st[:, 0:nb, :],
                    in_=Sv[:, s0 * WIN:(s0 + nb) * WIN].rearrange(
                        "p (a c) -> p a c", c=WIN))
                qi += 1
                for j in range(nb):
                    t = s0 + j
                    w, tw = divmod(t, T1)
                    a, q = divmod(w, 4)
                    dst = aggs[a // 16][32 * q:32 * q + 32, a % 16, 0:fdim]
                    nc.tensor.matmul(out=dst, lhsT=st[:, j, :],
                                     rhs=mt[:, j, 0:fdim],
                                     start=(tw == 0), stop=(tw == T1 - 1),
                                     tile_position=(0, 32 * q))
                for k, qd in enumerate(qdone):
                    if s0 < qd <= s0 + nb:
                        tail(k)
            tail(3)
    return nc


# ------------------------------------------------------------------ host prep


def _rank_within_group(group_sorted):
    n = group_sorted.shape[0]
    if n == 0:
        return np.zeros(0, dtype=np.int64)
    first = np.searchsorted(group_sorted, group_sorted, side="left")
    return np.arange(n, dtype=np.int64) - first


def plan_core(src, dst_local, u):
    E = src.shape[0]
    # gather side (src-sorted, 64-node chunks, flat 352-slot capacity)
    og = np.argsort(src, kind="stable")
    sg = src[og]
    chunk = sg // GCH
    rank = _rank_within_group(chunk)
    assert rank.max(initial=0) < CAPG, "gather chunk overflow"
    slot = chunk * CAPG + rank         # flat B column
    r = sg - chunk * GCH
    uo = u[og].astype(F32)
    Bt = np.zeros((P, N_SLOT), dtype=F8)
    Bt[r, slot] = (1.0 - uo).astype(F8)
    Bt[r + GCH, slot] = uo.astype(F8)
    # flat msg position: tiles grouped per chunk-batch by width class.
    # batches: (0,8), (8,24), then 32-chunk batches (matches build_gather).
    kcl = np.minimum(rank // 128, 2)
    row = rank - kcl * 128
    bi = np.searchsorted(_BST, chunk, side="right") - 1
    c0 = _BST[bi]
    ncb = _BSZ[bi]
    mtile = 3 * c0 + kcl * ncb + (chunk - c0)
    flat = mtile * P + row
    slot_of_edge = np.empty(E, dtype=np.int64)
    slot_of_edge[og] = flat
    # segsum side (dst-sorted, 32-node windows)
    os_ = np.argsort(dst_local, kind="stable")
    ds = dst_local[os_]
    win = ds // WIN
    rank_s = _rank_within_group(win)
    assert rank_s.max(initial=0) < CAPS, "segsum window overflow"
    pos = win * CAPS + rank_s          # == tile*128 + row
    st_ = pos // P
    sr = pos % P
    Sm = np.zeros((P, N_S * WIN), dtype=F8)
    Sm[sr, st_ * WIN + (ds - win * WIN)] = F8(1.0)
    perm = np.zeros((P, N_S), dtype=np.int64)
    perm[sr, st_] = slot_of_edge[os_]
    deg = np.bincount(dst_local, minlength=NPC).astype(F32)
    inv_pad = np.zeros(NPC_PAD, dtype=F32)
    inv_pad[:NPC] = 1.0 / np.clip(deg, 1.0, None)
    invd = np.ascontiguousarray(inv_pad.reshape(NT, P).T)
    return Bt, Sm, perm, invd


def _permute_msgs(gmsgs, perm, fdim):
    """gather msgs [P, NG*fdim] -> scatter-slot layout [P, N_S*fdim]."""
    flat = np.ascontiguousarray(
        gmsgs.reshape(P, NG, fdim).transpose(1, 0, 2)).reshape(NG * P, fdim)
    mp = flat[perm]                    # [P, N_S, fdim]
    return np.ascontiguousarray(mp).reshape(P, N_S * fdim)


def _stack_table(tglob, fdim):
    """[NPAD, 2*fdim] -> stacked gather table [P, N_CH_PAD*fdim]."""
    m = np.arange(NPAD)
    ck, ri = m // GCH, m % GCH
    tabS = np.zeros((P, N_CH_PAD, fdim), dtype=BF16)
    tabS[ri, ck] = tglob[:, 0:fdim]
    tabS[ri + GCH, ck] = tglob[:, fdim:2 * fdim]
    return np.ascontiguousarray(tabS).reshape(P, N_CH_PAD * fdim)


# ------------------------------------------------------------------ driver


_NC_CACHE = {}


def _get_nc(name, builder):
    if name not in _NC_CACHE:
        _NC_CACHE[name] = builder()
    return _NC_CACHE[name]


def _run(name, builder, in_maps):
    from concourse.bass_utils import run_bass_kernel_spmd
    import time
    nc = _get_nc(name, builder)
    t0 = time.time()
    res = run_bass_kernel_spmd(nc, in_maps, list(range(N_CORES)))
    _run.times[name] = time.time() - t0
    return res.results


_run.times = {}


def kernel(x, edge_attr, edge_index, W1, root1, b1, W2, root2, b2):
    import os
    dbg = bool(os.environ.get("KERNEL_DEBUG"))
    x = np.asarray(x, dtype=F32)
    u = np.asarray(edge_attr, dtype=F32).reshape(-1)
    ei = np.asarray(edge_index, dtype=np.int64)
    src_all, dst_all = ei[0], ei[1]

    # --- shard edges by dst owner core
    owner = dst_all // NPC
    plans = []
    for c in range(N_CORES):
        m = owner == c
        plans.append(plan_core(src_all[m], dst_all[m] - c * NPC, u[m]))

    # --- L1: GEMM (x @ [W1_0|W1_1|root1], bias row for root part)
    Wc = np.zeros((KPAD, 48), dtype=F32)
    Wc[:F_IN, 0:16] = np.asarray(W1[0], dtype=F32) * MSC
    Wc[:F_IN, 16:32] = np.asarray(W1[1], dtype=F32) * MSC
    Wc[:F_IN, 32:48] = np.asarray(root1, dtype=F32)
    Wc[F_IN, 32:48] = np.asarray(b1, dtype=F32)
    Wcp = np.ascontiguousarray(
        Wc.reshape(KT, P, 48).transpose(1, 0, 2)).reshape(P, KT * 48).astype(BF16)
    in1 = []
    for c in range(N_CORES):
        xf = np.zeros((NPC_PAD, KPAD), dtype=F8)
        xf[:NPC, :F_IN] = x[c * NPC:(c + 1) * NPC].astype(F8)
        xf[:NPC, F_IN] = F8(1.0)
        xTr = np.ascontiguousarray(
            xf.reshape(NT, P, KT, P).transpose(3, 0, 2, 1)).reshape(P, NT * KT * P)
        in1.append({"xTr": xTr, "Wcp": Wcp})
    r1 = _run("L1", build_L1, in1)
    tglob1 = np.zeros((NPAD, 32), dtype=BF16)
    roots = []
    for c in range(N_CORES):
        tl = r1[c]["table"].reshape(P, NT, 32).transpose(1, 0, 2).reshape(NPC_PAD, 32)
        tglob1[c * NPC:(c + 1) * NPC] = tl[:NPC]
        roots.append(np.ascontiguousarray(r1[c]["root"]))
    if dbg:
        xfull = np.zeros((N_NODES, KPAD), dtype=F32)
        xfull[:, :F_IN] = x
        xfull[:, F_IN] = 1.0
        Hexp = xfull @ Wc
        got = tglob1[:N_NODES].astype(F32)
        print("L1 table relerr:",
              np.abs(got - Hexp[:, 0:32]).max() / np.abs(Hexp[:, 0:32]).max())
        r0 = roots[0].reshape(P, NT, 16).transpose(1, 0, 2).reshape(NPC_PAD, 16)
        print("L1 root relerr:",
              np.abs(r0[:NPC] - Hexp[:NPC, 32:48]).max() / np.abs(Hexp[:, 32:48]).max())

    # --- L2: weighted gather layer 1
    tabS1 = _stack_table(tglob1, 16)
    in2 = [{"tabS": tabS1, "Bt": plans[c][0]} for c in range(N_CORES)]
    r2 = _run("L2", lambda: build_gather(16), in2)
    if dbg:
        c = 0
        m = owner == c
        s0, u0 = src_all[m], u[m]
        og = np.argsort(s0, kind="stable")
        sg = s0[og]
        ch = sg // GCH
        rk = _rank_within_group(ch)
        kcl = np.minimum(rk // 128, 2)
        bi = np.searchsorted(_BST, ch, side="right") - 1
        c0b = _BST[bi]
        ncb = _BSZ[bi]
        mtile = 3 * c0b + kcl * ncb + (ch - c0b)
        row = rk - kcl * 128
        tabf = tglob1.astype(F32)
        exp_msg = ((1 - u0[og])[:, None] * tabf[sg, 0:16]
                   + u0[og][:, None] * tabf[sg, 16:32])
        gm = r2[c]["msgs"].reshape(P, NG, 16)
        got = gm[row, mtile].astype(F32)
        print("L2 msg relerr:",
              np.abs(got - exp_msg).max() / np.abs(exp_msg).max())

    # --- L3: segsum + mean + root + ELU + GEMM2
    Wc2 = np.zeros((16, 21), dtype=BF16)
    Wc2[:, 0:7] = np.asarray(np.asarray(W2[0], dtype=F32) * MSC2, dtype=BF16)
    Wc2[:, 7:14] = np.asarray(np.asarray(W2[1], dtype=F32) * MSC2, dtype=BF16)
    Wc2[:, 14:21] = np.asarray(root2, dtype=BF16)
    b2row = np.zeros((1, 21), dtype=BF16)
    b2row[0, 14:21] = np.asarray(b2, dtype=BF16)
    ones1 = np.ones((1, P), dtype=BF16)
    id128 = np.eye(P, dtype=BF16)
    in3 = []
    for c in range(N_CORES):
        in3.append({"msgs": _permute_msgs(r2[c]["msgs"], plans[c][2], 16),
                    "S": plans[c][1], "invd": plans[c][3] / MSC,
                    "root": roots[c],
                    "Wc2": Wc2, "b2row": b2row, "ones1": ones1,
                    "id128": id128})
    r3 = _run("L3", lambda: build_segsum(16, True), in3)
    tglob2 = np.zeros((NPAD, 14), dtype=BF16)
    roots2 = []
    for c in range(N_CORES):
        tl = r3[c]["tab2"].reshape(P, NT, 14).transpose(1, 0, 2).reshape(NPC_PAD, 14)
        tglob2[c * NPC:(c + 1) * NPC] = tl[:NPC]
        roots2.append(np.ascontiguousarray(r3[c]["root2v"]))

    # --- L4: weighted gather layer 2
    tabS2 = _stack_table(tglob2, 7)
    in4 = [{"tabS": tabS2, "Bt": plans[c][0]} for c in range(N_CORES)]
    r4 = _run("L4", lambda: build_gather(7), in4)

    # --- L5: segsum + mean + root2 + log_softmax
    in5 = []
    for c in range(N_CORES):
        in5.append({"msgs": _permute_msgs(r4[c]["msgs"], plans[c][2], 7),
                    "S": plans[c][1], "invd": plans[c][3] / MSC2,
                    "root": roots2[c]})
    r5 = _run("L5", lambda: build_segsum(7, False), in5)

    out = np.zeros((N_NODES, F_OUT), dtype=F32)
    for c in range(N_CORES):
        ol = r5[c]["out"].reshape(P, NT, 7).transpose(1, 0, 2).reshape(NPC_PAD, 7)
        out[c * NPC:(c + 1) * NPC] = ol[:NPC]
    return out
